# revision 1
# baseline (speedup 1.0000x reference)
"""Bilinear sampler (spatial transformer) TRN2 Bass kernel.

Contract: kernel(inputs=[128, 196614] fp32) -> [128, 256, 256, 3] fp32.
Shards batch over 8 NeuronCores (16 images each). Per image on-device:
  - compute affine grid X = t00*j + t01*i + cx, Y likewise (ACT/DVE)
  - floors, bilinear weights with out-of-bounds masking (DVE)
  - build a row-pair interleaved copy of the image in DRAM scratch
    (site l = y*256+x holds rows y and y+1 of column x: 6 floats), so one
    contiguous 12-float fetch at offset 6*l yields the whole 2x2x3 patch
  - per pixel-column instruction: [P,1] indirect DMA gather (128 patches)
  - weighted blend of the 4 corners (DVE), DMA out
"""
import os
import sys

sys.path.insert(0, "/opt/trn_rl_repo")

import numpy as np

import concourse.bacc as bacc
import concourse.bass as bass
import concourse.mybir as mybir
import concourse.tile as tile
from concourse.bass_utils import run_bass_kernel_spmd

P = 128
H = W = 256
C = 3
IMG_ELS = H * W * C            # 196608
ROW_ELS = W * C                # 768
PW = (H * W) // P              # 512 pixels per partition per image
N_CORES = 8
IMGS = 16                      # images per core

F32 = mybir.dt.float32
I32 = mybir.dt.int32
ALU = mybir.AluOpType

_cached = {}


def _build(n_imgs):
    nc = bacc.Bacc("TRN2", target_bir_lowering=False, debug=False,
                   enable_asserts=False, num_devices=1, num_swdge_queues=1)
    inp = nc.dram_tensor("inp", [n_imgs, 6 + IMG_ELS], F32, kind="ExternalInput")
    xg_d = nc.dram_tensor("xg", [P, PW], F32, kind="ExternalInput")
    yg_d = nc.dram_tensor("yg", [P, PW], F32, kind="ExternalInput")
    cst_d = nc.dram_tensor("cst", [2, 4], F32, kind="ExternalInput")
    out_d = nc.dram_tensor("out", [n_imgs, H * W * C], F32, kind="ExternalOutput")
    idups = [nc.dram_tensor(f"idup{b}", [H * W, 6], F32) for b in range(n_imgs)]
    scr = nc.dram_tensor("scr", [n_imgs, 8], F32)

    with tile.TileContext(nc) as tc:
        with (
            tc.tile_pool(name="const", bufs=1) as cpool,
            tc.tile_pool(name="work", bufs=1) as wp,
            tc.tile_pool(name="gath", bufs=2) as gpool,
            tc.tile_pool(name="offp", bufs=2) as opool,
            tc.tile_pool(name="wgt", bufs=2) as wpool,
        ):
            xg = cpool.tile([P, PW], F32)
            nc.sync.dma_start(xg[:], xg_d[:, :])
            yg = cpool.tile([P, PW], F32)
            nc.sync.dma_start(yg[:], yg_d[:, :])
            cst = cpool.tile([2, 4], F32)
            nc.sync.dma_start(cst[:], cst_d[:, :])

            def bc3(t):
                return bass.AP(t.tensor, t.offset, list(t.ap) + [[0, 3]])

            def blend_store(st):
                # slices (r,s): 0:3=(0,0) 3:6=(1,0) 6:9=(0,1) 9:12=(1,1)
                pg, pw00, pw10, pw01, pw11, pb = st
                t0 = wp.tile([P, PW, 3], F32, tag="bl_t0")
                nc.vector.tensor_tensor(out=t0[:], in0=pg[:, :, 0:3],
                                        in1=bc3(pw00[:]), op=ALU.mult)
                t1 = wp.tile([P, PW, 3], F32, tag="bl_t1")
                nc.vector.tensor_tensor(out=t1[:], in0=pg[:, :, 3:6],
                                        in1=bc3(pw10[:]), op=ALU.mult)
                t2 = wp.tile([P, PW, 3], F32, tag="bl_t2")
                nc.vector.tensor_tensor(out=t2[:], in0=pg[:, :, 6:9],
                                        in1=bc3(pw01[:]), op=ALU.mult)
                t3 = wp.tile([P, PW, 3], F32, tag="bl_t3")
                nc.vector.tensor_tensor(out=t3[:], in0=pg[:, :, 9:12],
                                        in1=bc3(pw11[:]), op=ALU.mult)
                nc.vector.tensor_tensor(out=t0[:], in0=t0[:], in1=t1[:], op=ALU.add)
                nc.vector.tensor_tensor(out=t2[:], in0=t2[:], in1=t3[:], op=ALU.add)
                ob = wp.tile([P, PW, 3], F32, tag="bl_ob")
                nc.vector.tensor_tensor(out=ob[:], in0=t0[:], in1=t2[:], op=ALU.add)
                nc.sync.dma_start(
                    bass.AP(out_d, pb * IMG_ELS, [[PW * 3, P], [1, PW * 3]]),
                    ob[:])

            prev = None
            for b in range(n_imgs):
                # ---- affine params: [2,3] theta rows; cx/cy = 127.5*(t2+1-t0-t1)
                th = wp.tile([2, 3], F32)
                nc.sync.dma_start(th[:], bass.AP(inp, b * (6 + IMG_ELS), [[3, 2], [1, 3]]))
                m = wp.tile([2, 3], F32)
                nc.vector.tensor_tensor(out=m[:], in0=th[:], in1=cst[:, 0:3], op=ALU.mult)
                s = wp.tile([2, 1], F32)
                nc.vector.tensor_reduce(out=s[:], in_=m[:], axis=mybir.AxisListType.X, op=ALU.add)
                pr = wp.tile([2, 4], F32)
                nc.vector.tensor_copy(out=pr[:, 0:3], in_=th[:])
                nc.vector.tensor_scalar(out=pr[:, 3:4], in0=s[:], scalar1=127.5,
                                        scalar2=None, op0=ALU.add)
                nc.sync.dma_start(bass.AP(scr, b * 8, [[4, 2], [1, 4]]), pr[:])
                thb = wp.tile([P, 8], F32)
                nc.sync.dma_start(thb[:], bass.AP(scr, b * 8, [[0, P], [1, 8]]))
                # thb cols: 0=t00 1=t01 2=t02(unused) 3=cx 4=t10 5=t11 6=t12 7=cy

                # ---- build row-pair interleaved image copy in DRAM
                it = wp.tile([P, 1536], F32)
                nc.sync.dma_start(it[:], bass.AP(inp, b * (6 + IMG_ELS) + 6,
                                                 [[1536, P], [1, 1536]]))
                hal = wp.tile([P, ROW_ELS], F32)
                nc.sync.dma_start(hal[0:127, :],
                                  bass.AP(inp, b * (6 + IMG_ELS) + 6 + 1536,
                                          [[1536, 127], [1, ROW_ELS]]))
                nc.sync.dma_start(hal[127:128, :],
                                  bass.AP(inp, b * (6 + IMG_ELS) + 6 + IMG_ELS - ROW_ELS,
                                          [[ROW_ELS, 1], [1, ROW_ELS]]))
                d2 = wp.tile([P, PW, 6], F32)
                it3 = it[:].rearrange("p (w c) -> p w c", c=3)
                nc.vector.tensor_copy(out=d2[:, :, 0:3], in_=it3)
                nc.vector.tensor_copy(out=d2[:, 0:256, 3:6],
                                      in_=it[:, ROW_ELS:1536].rearrange("p (w c) -> p w c", c=3))
                nc.vector.tensor_copy(out=d2[:, 256:512, 3:6],
                                      in_=hal[:].rearrange("p (w c) -> p w c", c=3))
                nc.sync.dma_start(idups[b][:, :], d2[:])

                # ---- grid coords
                X = wp.tile([P, PW], F32)
                nc.vector.tensor_scalar(out=X[:], in0=xg[:], scalar1=thb[:, 0:1],
                                        scalar2=None, op0=ALU.mult)
                X2 = wp.tile([P, PW], F32)
                nc.vector.scalar_tensor_tensor(out=X2[:], in0=yg[:], scalar=thb[:, 1:2],
                                               in1=X[:], op0=ALU.mult, op1=ALU.add)
                nc.vector.tensor_scalar(out=X[:], in0=X2[:], scalar1=thb[:, 3:4],
                                        scalar2=None, op0=ALU.add)
                Y = wp.tile([P, PW], F32)
                nc.vector.tensor_scalar(out=Y[:], in0=xg[:], scalar1=thb[:, 4:5],
                                        scalar2=None, op0=ALU.mult)
                Y2 = wp.tile([P, PW], F32)
                nc.vector.scalar_tensor_tensor(out=Y2[:], in0=yg[:], scalar=thb[:, 5:6],
                                               in1=Y[:], op0=ALU.mult, op1=ALU.add)
                nc.vector.tensor_scalar(out=Y[:], in0=Y2[:], scalar1=thb[:, 7:8],
                                        scalar2=None, op0=ALU.add)

                # ---- floor via int truncation + correction
                def floor_of(src, nm):
                    ti = wp.tile([P, PW], I32, tag=f"fl_i{nm}")
                    nc.vector.tensor_copy(out=ti[:], in_=src[:])
                    tf = wp.tile([P, PW], F32, tag=f"fl_f{nm}")
                    nc.vector.tensor_copy(out=tf[:], in_=ti[:])
                    gt = wp.tile([P, PW], F32, tag=f"fl_g{nm}")
                    nc.vector.tensor_tensor(out=gt[:], in0=tf[:], in1=src[:], op=ALU.is_gt)
                    fl = wp.tile([P, PW], F32, tag=f"fl_o{nm}")
                    nc.vector.tensor_tensor(out=fl[:], in0=tf[:], in1=gt[:], op=ALU.subtract)
                    return fl

                xf = floor_of(X, "x")
                yf = floor_of(Y, "y")

                # ---- weights with OOB masks
                fx = wp.tile([P, PW], F32)
                nc.vector.tensor_tensor(out=fx[:], in0=X[:], in1=xf[:], op=ALU.subtract)
                fy = wp.tile([P, PW], F32)
                nc.vector.tensor_tensor(out=fy[:], in0=Y[:], in1=yf[:], op=ALU.subtract)
                al = wp.tile([P, PW], F32)
                nc.vector.tensor_scalar(out=al[:], in0=fx[:], scalar1=-1.0, scalar2=1.0,
                                        op0=ALU.mult, op1=ALU.add)
                ga = wp.tile([P, PW], F32)
                nc.vector.tensor_scalar(out=ga[:], in0=fy[:], scalar1=-1.0, scalar2=1.0,
                                        op0=ALU.mult, op1=ALU.add)
                mgx = wp.tile([P, PW], F32)
                nc.vector.tensor_scalar(out=mgx[:], in0=xf[:], scalar1=0.0, scalar2=None,
                                        op0=ALU.is_ge)
                mx = wp.tile([P, PW], F32)
                nc.vector.scalar_tensor_tensor(out=mx[:], in0=xf[:], scalar=254.0,
                                               in1=mgx[:], op0=ALU.is_le, op1=ALU.mult)
                mgy = wp.tile([P, PW], F32)
                nc.vector.tensor_scalar(out=mgy[:], in0=yf[:], scalar1=0.0, scalar2=None,
                                        op0=ALU.is_ge)
                my = wp.tile([P, PW], F32)
                nc.vector.scalar_tensor_tensor(out=my[:], in0=yf[:], scalar=254.0,
                                               in1=mgy[:], op0=ALU.is_le, op1=ALU.mult)
                A = wp.tile([P, PW], F32)
                nc.vector.tensor_tensor(out=A[:], in0=al[:], in1=mx[:], op=ALU.mult)
                Bw = wp.tile([P, PW], F32)
                nc.vector.tensor_tensor(out=Bw[:], in0=fx[:], in1=mx[:], op=ALU.mult)
                Cw = wp.tile([P, PW], F32)
                nc.vector.tensor_tensor(out=Cw[:], in0=ga[:], in1=my[:], op=ALU.mult)
                Dw = wp.tile([P, PW], F32)
                nc.vector.tensor_tensor(out=Dw[:], in0=fy[:], in1=my[:], op=ALU.mult)
                w00 = wpool.tile([P, PW], F32)
                nc.vector.tensor_tensor(out=w00[:], in0=Cw[:], in1=A[:], op=ALU.mult)
                w10 = wpool.tile([P, PW], F32)
                nc.vector.tensor_tensor(out=w10[:], in0=Dw[:], in1=A[:], op=ALU.mult)
                w01 = wpool.tile([P, PW], F32)
                nc.vector.tensor_tensor(out=w01[:], in0=Cw[:], in1=Bw[:], op=ALU.mult)
                w11 = wpool.tile([P, PW], F32)
                nc.vector.tensor_tensor(out=w11[:], in0=Dw[:], in1=Bw[:], op=ALU.mult)

                # ---- gather offsets: site = clamp(yf,0,254)*256 + clamp(xf,0,254)
                xc = wp.tile([P, PW], F32)
                nc.vector.tensor_scalar(out=xc[:], in0=xf[:], scalar1=0.0, scalar2=254.0,
                                        op0=ALU.max, op1=ALU.min)
                yc = wp.tile([P, PW], F32)
                nc.vector.tensor_scalar(out=yc[:], in0=yf[:], scalar1=0.0, scalar2=254.0,
                                        op0=ALU.max, op1=ALU.min)
                lf = wp.tile([P, PW], F32)
                nc.vector.scalar_tensor_tensor(out=lf[:], in0=yc[:], scalar=256.0,
                                               in1=xc[:], op0=ALU.mult, op1=ALU.add)
                off = opool.tile([P, PW], I32)
                nc.vector.tensor_copy(out=off[:], in_=lf[:])

                # ---- per-column [P,1] patch gathers, single SWDGE queue
                # (queue alternation costs per-instruction ring bookkeeping on
                # the Pool ucode; the DMA engines are only ~10% busy, so the
                # second queue adds no transfer parallelism worth having)
                g = gpool.tile([P, PW, 12], F32)
                for w in range(PW):
                    nc.gpsimd.indirect_dma_start(
                        out=g[:, w, :], out_offset=None,
                        in_=idups[b][:, :],
                        in_offset=bass.IndirectOffsetOnAxis(ap=off[:, w:w + 1], axis=0))

                # blend lags one image so image b+1's DVE prework overlaps
                # image b's gather stream on the Pool engine
                if prev is not None:
                    blend_store(prev)
                prev = (g, w00, w10, w01, w11, b)

            blend_store(prev)
    nc.compile()
    return nc


def _consts():
    # pixel (p, w): global l = p*PW + w ; j = l % 256 ; i = l // 256
    l = np.arange(P * PW).reshape(P, PW)
    xg = (l % 256).astype(np.float32)
    yg = (l // 256).astype(np.float32)
    cst = np.tile(np.array([-127.5, -127.5, 127.5, 0.0], np.float32), (2, 1))
    return xg, yg, cst


IMGS_PER_LAUNCH = 16


def kernel(inputs: np.ndarray) -> np.ndarray:
    inputs = np.ascontiguousarray(inputs, dtype=np.float32)
    assert inputs.shape == (128, 6 + IMG_ELS)
    npl = IMGS_PER_LAUNCH
    if npl not in _cached:
        _cached[npl] = _build(npl)
    nc = _cached[npl]
    xg, yg, cst = _consts()
    trace = bool(os.environ.get("BILIN_TRACE"))
    if trace:
        try:  # NTFF trace hook is missing from this image's antenv; install shim
            import antenv.axon_hooks  # noqa: F401
        except ImportError:
            try:
                import types
                from trn_agent_boot.trn_boot import _ntff_profile_via_ctypes
                hook = _ntff_profile_via_ctypes("/opt/axon/libaxon_pjrt.so")
                mod = types.ModuleType("antenv.axon_hooks")
                mod.get_axon_ntff_profile_hook = lambda: hook
                sys.modules["antenv.axon_hooks"] = mod
            except Exception:
                trace = False
    out = np.empty((128, H, W, C), np.float32)
    total_ns = 0
    n_launches = IMGS // npl
    for k in range(n_launches):
        in_maps = []
        for c in range(N_CORES):
            lo = c * IMGS + k * npl
            in_maps.append(dict(inp=np.ascontiguousarray(inputs[lo:lo + npl]),
                                xg=xg, yg=yg, cst=cst))
        res = run_bass_kernel_spmd(nc, in_maps, core_ids=list(range(N_CORES)),
                                   trace=trace and k == 0)
        if trace and k == 0 and res.exec_time_ns is not None:
            total_ns = res.exec_time_ns * n_launches
        for c in range(N_CORES):
            lo = c * IMGS + k * npl
            out[lo:lo + npl] = res.results[c]["out"].reshape(npl, H, W, C)
    if trace:
        print(f"HW exec time: {total_ns} ns")
    return out



# revision 7
# speedup vs baseline: 2.0091x; 2.0091x over previous
"""Bilinear sampler (spatial transformer) TRN2 Bass kernel.

Contract: kernel(inputs=[128, 196614] fp32) -> [128, 256, 256, 3] fp32.
Shards batch over 8 NeuronCores (16 images each). Per image on-device:
  - compute affine grid X = t00*j + t01*i + cx, Y likewise (ACT/DVE)
  - floors, bilinear weights with out-of-bounds masking (DVE)
  - build a row-pair interleaved copy of the image in DRAM scratch
    (site l = y*256+x holds rows y and y+1 of column x: 6 floats), so one
    contiguous 12-float fetch at offset 6*l yields the whole 2x2x3 patch
  - per pixel-column instruction: [P,1] indirect DMA gather (128 patches)
  - weighted blend of the 4 corners (DVE), DMA out
"""
import os
import sys

sys.path.insert(0, "/opt/trn_rl_repo")

import numpy as np

import concourse.bacc as bacc
import concourse.bass as bass
import concourse.mybir as mybir
import concourse.tile as tile
from concourse.bass_utils import run_bass_kernel_spmd

P = 128
H = W = 256
C = 3
IMG_ELS = H * W * C            # 196608
ROW_ELS = W * C                # 768
PW = (H * W) // P              # 512 pixels per partition per image
N_CORES = 8
IMGS = 16                      # images per core

F32 = mybir.dt.float32
I32 = mybir.dt.int32
ALU = mybir.AluOpType

_cached = {}


def _build(n_imgs):
    nc = bacc.Bacc("TRN2", target_bir_lowering=False, debug=False,
                   enable_asserts=False, num_devices=1, num_swdge_queues=1)
    inp = nc.dram_tensor("inp", [n_imgs, 6 + IMG_ELS], F32, kind="ExternalInput")
    xg_d = nc.dram_tensor("xg", [P, PW], F32, kind="ExternalInput")
    yg_d = nc.dram_tensor("yg", [P, PW], F32, kind="ExternalInput")
    cst_d = nc.dram_tensor("cst", [2, 4], F32, kind="ExternalInput")
    out_d = nc.dram_tensor("out", [n_imgs, H * W * C], F32, kind="ExternalOutput")
    idups = [nc.dram_tensor(f"idup{b}", [H * W, 6], F32) for b in range(n_imgs)]
    scr = nc.dram_tensor("scr", [n_imgs, 8], F32)

    with tile.TileContext(nc) as tc:
        with (
            tc.tile_pool(name="const", bufs=1) as cpool,
            tc.tile_pool(name="work", bufs=1) as wp,
            tc.tile_pool(name="gath", bufs=2) as gpool,
            tc.tile_pool(name="offp", bufs=2) as opool,
            tc.tile_pool(name="wgt", bufs=2) as wpool,
        ):
            xg = cpool.tile([P, PW], F32)
            nc.sync.dma_start(xg[:], xg_d[:, :])
            yg = cpool.tile([P, PW], F32)
            nc.sync.dma_start(yg[:], yg_d[:, :])
            cst = cpool.tile([2, 4], F32)
            nc.sync.dma_start(cst[:], cst_d[:, :])

            def bc3(t):
                return bass.AP(t.tensor, t.offset, list(t.ap) + [[0, 3]])

            def blend_store(st):
                # slices (r,s): 0:3=(0,0) 3:6=(1,0) 6:9=(0,1) 9:12=(1,1)
                pg, pw00, pw10, pw01, pw11, pb = st
                t0 = wp.tile([P, PW, 3], F32, tag="bl_t0")
                nc.vector.tensor_tensor(out=t0[:], in0=pg[:, :, 0:3],
                                        in1=bc3(pw00[:]), op=ALU.mult)
                t1 = wp.tile([P, PW, 3], F32, tag="bl_t1")
                nc.vector.tensor_tensor(out=t1[:], in0=pg[:, :, 3:6],
                                        in1=bc3(pw10[:]), op=ALU.mult)
                t2 = wp.tile([P, PW, 3], F32, tag="bl_t2")
                nc.vector.tensor_tensor(out=t2[:], in0=pg[:, :, 6:9],
                                        in1=bc3(pw01[:]), op=ALU.mult)
                t3 = wp.tile([P, PW, 3], F32, tag="bl_t3")
                nc.vector.tensor_tensor(out=t3[:], in0=pg[:, :, 9:12],
                                        in1=bc3(pw11[:]), op=ALU.mult)
                nc.vector.tensor_tensor(out=t0[:], in0=t0[:], in1=t1[:], op=ALU.add)
                nc.vector.tensor_tensor(out=t2[:], in0=t2[:], in1=t3[:], op=ALU.add)
                ob = wp.tile([P, PW, 3], F32, tag="bl_ob")
                nc.vector.tensor_tensor(out=ob[:], in0=t0[:], in1=t2[:], op=ALU.add)
                nc.sync.dma_start(
                    bass.AP(out_d, pb * IMG_ELS, [[PW * 3, P], [1, PW * 3]]),
                    ob[:])

            prev = None
            for b in range(n_imgs):
                # ---- affine params: [2,3] theta rows; cx/cy = 127.5*(t2+1-t0-t1)
                th = wp.tile([2, 3], F32)
                nc.sync.dma_start(th[:], bass.AP(inp, b * (6 + IMG_ELS), [[3, 2], [1, 3]]))
                m = wp.tile([2, 3], F32)
                nc.vector.tensor_tensor(out=m[:], in0=th[:], in1=cst[:, 0:3], op=ALU.mult)
                s = wp.tile([2, 1], F32)
                nc.vector.tensor_reduce(out=s[:], in_=m[:], axis=mybir.AxisListType.X, op=ALU.add)
                pr = wp.tile([2, 4], F32)
                nc.vector.tensor_copy(out=pr[:, 0:3], in_=th[:])
                nc.vector.tensor_scalar(out=pr[:, 3:4], in0=s[:], scalar1=127.5,
                                        scalar2=None, op0=ALU.add)
                nc.sync.dma_start(bass.AP(scr, b * 8, [[4, 2], [1, 4]]), pr[:])
                thb = wp.tile([P, 8], F32)
                nc.sync.dma_start(thb[:], bass.AP(scr, b * 8, [[0, P], [1, 8]]))
                # thb cols: 0=t00 1=t01 2=t02(unused) 3=cx 4=t10 5=t11 6=t12 7=cy

                # ---- build row-pair interleaved image copy in DRAM
                it = wp.tile([P, 1536], F32)
                nc.sync.dma_start(it[:], bass.AP(inp, b * (6 + IMG_ELS) + 6,
                                                 [[1536, P], [1, 1536]]))
                hal = wp.tile([P, ROW_ELS], F32)
                nc.sync.dma_start(hal[0:127, :],
                                  bass.AP(inp, b * (6 + IMG_ELS) + 6 + 1536,
                                          [[1536, 127], [1, ROW_ELS]]))
                nc.sync.dma_start(hal[127:128, :],
                                  bass.AP(inp, b * (6 + IMG_ELS) + 6 + IMG_ELS - ROW_ELS,
                                          [[ROW_ELS, 1], [1, ROW_ELS]]))
                d2 = wp.tile([P, PW, 6], F32)
                it3 = it[:].rearrange("p (w c) -> p w c", c=3)
                nc.vector.tensor_copy(out=d2[:, :, 0:3], in_=it3)
                nc.vector.tensor_copy(out=d2[:, 0:256, 3:6],
                                      in_=it[:, ROW_ELS:1536].rearrange("p (w c) -> p w c", c=3))
                nc.vector.tensor_copy(out=d2[:, 256:512, 3:6],
                                      in_=hal[:].rearrange("p (w c) -> p w c", c=3))
                nc.sync.dma_start(idups[b][:, :], d2[:])

                # ---- grid coords
                X = wp.tile([P, PW], F32)
                nc.vector.tensor_scalar(out=X[:], in0=xg[:], scalar1=thb[:, 0:1],
                                        scalar2=None, op0=ALU.mult)
                X2 = wp.tile([P, PW], F32)
                nc.vector.scalar_tensor_tensor(out=X2[:], in0=yg[:], scalar=thb[:, 1:2],
                                               in1=X[:], op0=ALU.mult, op1=ALU.add)
                nc.vector.tensor_scalar(out=X[:], in0=X2[:], scalar1=thb[:, 3:4],
                                        scalar2=None, op0=ALU.add)
                Y = wp.tile([P, PW], F32)
                nc.vector.tensor_scalar(out=Y[:], in0=xg[:], scalar1=thb[:, 4:5],
                                        scalar2=None, op0=ALU.mult)
                Y2 = wp.tile([P, PW], F32)
                nc.vector.scalar_tensor_tensor(out=Y2[:], in0=yg[:], scalar=thb[:, 5:6],
                                               in1=Y[:], op0=ALU.mult, op1=ALU.add)
                nc.vector.tensor_scalar(out=Y[:], in0=Y2[:], scalar1=thb[:, 7:8],
                                        scalar2=None, op0=ALU.add)

                # ---- floor via int truncation + correction
                def floor_of(src, nm):
                    ti = wp.tile([P, PW], I32, tag=f"fl_i{nm}")
                    nc.vector.tensor_copy(out=ti[:], in_=src[:])
                    tf = wp.tile([P, PW], F32, tag=f"fl_f{nm}")
                    nc.vector.tensor_copy(out=tf[:], in_=ti[:])
                    gt = wp.tile([P, PW], F32, tag=f"fl_g{nm}")
                    nc.vector.tensor_tensor(out=gt[:], in0=tf[:], in1=src[:], op=ALU.is_gt)
                    fl = wp.tile([P, PW], F32, tag=f"fl_o{nm}")
                    nc.vector.tensor_tensor(out=fl[:], in0=tf[:], in1=gt[:], op=ALU.subtract)
                    return fl

                xf = floor_of(X, "x")
                yf = floor_of(Y, "y")

                # ---- weights with OOB masks
                fx = wp.tile([P, PW], F32)
                nc.vector.tensor_tensor(out=fx[:], in0=X[:], in1=xf[:], op=ALU.subtract)
                fy = wp.tile([P, PW], F32)
                nc.vector.tensor_tensor(out=fy[:], in0=Y[:], in1=yf[:], op=ALU.subtract)
                al = wp.tile([P, PW], F32)
                nc.vector.tensor_scalar(out=al[:], in0=fx[:], scalar1=-1.0, scalar2=1.0,
                                        op0=ALU.mult, op1=ALU.add)
                ga = wp.tile([P, PW], F32)
                nc.vector.tensor_scalar(out=ga[:], in0=fy[:], scalar1=-1.0, scalar2=1.0,
                                        op0=ALU.mult, op1=ALU.add)
                mgx = wp.tile([P, PW], F32)
                nc.vector.tensor_scalar(out=mgx[:], in0=xf[:], scalar1=0.0, scalar2=None,
                                        op0=ALU.is_ge)
                mx = wp.tile([P, PW], F32)
                nc.vector.scalar_tensor_tensor(out=mx[:], in0=xf[:], scalar=254.0,
                                               in1=mgx[:], op0=ALU.is_le, op1=ALU.mult)
                mgy = wp.tile([P, PW], F32)
                nc.vector.tensor_scalar(out=mgy[:], in0=yf[:], scalar1=0.0, scalar2=None,
                                        op0=ALU.is_ge)
                my = wp.tile([P, PW], F32)
                nc.vector.scalar_tensor_tensor(out=my[:], in0=yf[:], scalar=254.0,
                                               in1=mgy[:], op0=ALU.is_le, op1=ALU.mult)
                A = wp.tile([P, PW], F32)
                nc.vector.tensor_tensor(out=A[:], in0=al[:], in1=mx[:], op=ALU.mult)
                Bw = wp.tile([P, PW], F32)
                nc.vector.tensor_tensor(out=Bw[:], in0=fx[:], in1=mx[:], op=ALU.mult)
                Cw = wp.tile([P, PW], F32)
                nc.vector.tensor_tensor(out=Cw[:], in0=ga[:], in1=my[:], op=ALU.mult)
                Dw = wp.tile([P, PW], F32)
                nc.vector.tensor_tensor(out=Dw[:], in0=fy[:], in1=my[:], op=ALU.mult)
                w00 = wpool.tile([P, PW], F32)
                nc.vector.tensor_tensor(out=w00[:], in0=Cw[:], in1=A[:], op=ALU.mult)
                w10 = wpool.tile([P, PW], F32)
                nc.vector.tensor_tensor(out=w10[:], in0=Dw[:], in1=A[:], op=ALU.mult)
                w01 = wpool.tile([P, PW], F32)
                nc.vector.tensor_tensor(out=w01[:], in0=Cw[:], in1=Bw[:], op=ALU.mult)
                w11 = wpool.tile([P, PW], F32)
                nc.vector.tensor_tensor(out=w11[:], in0=Dw[:], in1=Bw[:], op=ALU.mult)

                # ---- gather offsets: site = clamp(yf,0,254)*256 + clamp(xf,0,254)
                xc = wp.tile([P, PW], F32)
                nc.vector.tensor_scalar(out=xc[:], in0=xf[:], scalar1=0.0, scalar2=254.0,
                                        op0=ALU.max, op1=ALU.min)
                yc = wp.tile([P, PW], F32)
                nc.vector.tensor_scalar(out=yc[:], in0=yf[:], scalar1=0.0, scalar2=254.0,
                                        op0=ALU.max, op1=ALU.min)
                lf = wp.tile([P, PW], F32)
                nc.vector.scalar_tensor_tensor(out=lf[:], in0=yc[:], scalar=256.0,
                                               in1=xc[:], op0=ALU.mult, op1=ALU.add)
                off = opool.tile([P, PW], I32)
                nc.vector.tensor_copy(out=off[:], in_=lf[:])

                # ---- per-column [P,1] patch gathers, single SWDGE queue
                # (queue alternation costs per-instruction ring bookkeeping on
                # the Pool ucode; the DMA engines are only ~10% busy, so the
                # second queue adds no transfer parallelism worth having)
                g = gpool.tile([P, PW, 12], F32)
                for w in range(PW):
                    nc.gpsimd.indirect_dma_start(
                        out=g[:, w, :], out_offset=None,
                        in_=idups[b][:, :],
                        in_offset=bass.IndirectOffsetOnAxis(ap=off[:, w:w + 1], axis=0))

                # blend lags one image so image b+1's DVE prework overlaps
                # image b's gather stream on the Pool engine
                if prev is not None:
                    blend_store(prev)
                prev = (g, w00, w10, w01, w11, b)

            blend_store(prev)
    nc.compile()
    return nc


def _consts():
    # pixel (p, w): global l = p*PW + w ; j = l % 256 ; i = l // 256
    l = np.arange(P * PW).reshape(P, PW)
    xg = (l % 256).astype(np.float32)
    yg = (l // 256).astype(np.float32)
    cst = np.tile(np.array([-127.5, -127.5, 127.5, 0.0], np.float32), (2, 1))
    return xg, yg, cst


IMGS_PER_LAUNCH = 16


def kernel(inputs: np.ndarray) -> np.ndarray:
    inputs = np.ascontiguousarray(inputs, dtype=np.float32)
    assert inputs.shape == (128, 6 + IMG_ELS)
    npl = IMGS_PER_LAUNCH
    if npl not in _cached:
        _cached[npl] = _build(npl)
    nc = _cached[npl]
    xg, yg, cst = _consts()
    trace = bool(os.environ.get("BILIN_TRACE"))
    if trace:
        try:  # NTFF trace hook is missing from this image's antenv; install shim
            import antenv.axon_hooks  # noqa: F401
        except ImportError:
            try:
                import types
                from trn_agent_boot.trn_boot import _ntff_profile_via_ctypes
                hook = _ntff_profile_via_ctypes("/opt/axon/libaxon_pjrt.so")
                mod = types.ModuleType("antenv.axon_hooks")
                mod.get_axon_ntff_profile_hook = lambda: hook
                sys.modules["antenv.axon_hooks"] = mod
            except Exception:
                trace = False
    out = np.empty((128, H, W, C), np.float32)
    total_ns = 0
    n_launches = IMGS // npl
    for k in range(n_launches):
        in_maps = []
        for c in range(N_CORES):
            lo = c * IMGS + k * npl
            in_maps.append(dict(inp=np.ascontiguousarray(inputs[lo:lo + npl]),
                                xg=xg, yg=yg, cst=cst))
        res = run_bass_kernel_spmd(nc, in_maps, core_ids=list(range(N_CORES)),
                                   trace=trace and k == 0)
        if trace and k == 0 and res.exec_time_ns is not None:
            total_ns = res.exec_time_ns * n_launches
        for c in range(N_CORES):
            lo = c * IMGS + k * npl
            out[lo:lo + npl] = res.results[c]["out"].reshape(npl, H, W, C)
    if trace:
        print(f"HW exec time: {total_ns} ns")
    return out


# revision 8
# speedup vs baseline: 2.0113x; 1.0011x over previous
"""Bilinear sampler TRN2 kernel, v2: interval-packed [P,1] gathers.

Key idea: the in-bounds output region (nonzero weights) is the preimage of
the source square under the affine map — per output row i an interval
[j_lo(i), j_hi(i)].  All other pixels are exactly zero in the reference
(clipped-corner weights cancel).  The kernel is value-specialized per call:
the host derives, from the 6 affine params per image (coordinates only, no
image data), a per-row gather window, packs one row per partition with a
per-partition shift, and the device gathers only max_p(len) columns per
pass instead of 512.  Two passes of 128 rows cover the image; results are
scattered back with a single [P,1] indirect DMA per pass into a zero-filled
output.  Per-image instruction count drops ~2.3x vs the dense kernel;
the Pool engine (994ns+ per SWDGE indirect instruction) is the bottleneck.

Layout per pass s of image b: partition p handles output row r(p) =
rows_sorted_by_len[s*128+p], columns j = D(p) + w for w in [0, Wslot).
X = t00*(D+w) + t01*r + cx = t00*w + A(p), A host-computed per partition.
"""
import os
import sys

sys.path.insert(0, "/opt/trn_rl_repo")

import numpy as np

import concourse.bacc as bacc
import concourse.bass as bass
import concourse.mybir as mybir
import concourse.tile as tile
from concourse.bass_utils import run_bass_kernel_spmd

P = 128
H = W = 256
C = 3
IMG_ELS = H * W * C            # 196608
ROW_ELS = W * C                # 768
N_CORES = 8
IMGS = 16
PAD = 2                        # interval padding vs host float rounding
WMAX = 256

F32 = mybir.dt.float32
I32 = mybir.dt.int32
ALU = mybir.AluOpType

_cached = {}


def _build(wslots):
    """wslots: tuple of 16 (W_A, W_B) per image slot."""
    nc = bacc.Bacc("TRN2", target_bir_lowering=False, debug=False,
                   enable_asserts=False, num_devices=1, num_swdge_queues=1)
    n_imgs = len(wslots)
    inp = nc.dram_tensor("inp", [n_imgs, 6 + IMG_ELS], F32, kind="ExternalInput")
    wio_d = nc.dram_tensor("wio", [P, WMAX], F32, kind="ExternalInput")
    th4_d = nc.dram_tensor("th4", [n_imgs * 2 * P, 4], F32, kind="ExternalInput")
    sct_d = nc.dram_tensor("sct", [n_imgs * 2 * P, 1], I32, kind="ExternalInput")
    out_d = nc.dram_tensor("outb", [n_imgs * H * W, C], F32, kind="ExternalOutput")
    idups = [nc.dram_tensor(f"idup{b}", [H * W, 6], F32) for b in range(n_imgs)]

    with tile.TileContext(nc) as tc:
        with (
            tc.tile_pool(name="const", bufs=1) as cpool,
            tc.tile_pool(name="work", bufs=1) as wp,
            tc.tile_pool(name="d2p", bufs=2) as dp,
            tc.tile_pool(name="gath", bufs=3) as gpool,
            tc.tile_pool(name="wgt", bufs=2) as wpool,
            tc.tile_pool(name="outp", bufs=2) as opool,
        ):
            wio = cpool.tile([P, WMAX], F32)
            nc.sync.dma_start(wio[:], wio_d[:, :])
            zt = cpool.tile([P, 1536], F32)
            nc.vector.memset(zt[:], 0.0)

            def bc3(ap):
                return bass.AP(ap.tensor, ap.offset, list(ap.ap) + [[0, 3]])

            def blend_scatter(st):
                pg, pw00, pw10, pw01, pw11, psct, wn = st
                t0 = opool.tile([P, WMAX, 3], F32, tag="bl_t0")
                t1 = opool.tile([P, WMAX, 3], F32, tag="bl_t1")
                a0 = t0[:, 0:wn, :]
                a1 = t1[:, 0:wn, :]
                nc.vector.tensor_tensor(out=a0, in0=pg[:, 0:wn, 0:3],
                                        in1=bc3(pw00[:, 0:wn]), op=ALU.mult)
                nc.vector.tensor_tensor(out=a1, in0=pg[:, 0:wn, 3:6],
                                        in1=bc3(pw10[:, 0:wn]), op=ALU.mult)
                nc.vector.tensor_tensor(out=a0, in0=a0, in1=a1, op=ALU.add)
                nc.vector.tensor_tensor(out=a1, in0=pg[:, 0:wn, 6:9],
                                        in1=bc3(pw01[:, 0:wn]), op=ALU.mult)
                nc.vector.tensor_tensor(out=a0, in0=a0, in1=a1, op=ALU.add)
                nc.vector.tensor_tensor(out=a1, in0=pg[:, 0:wn, 9:12],
                                        in1=bc3(pw11[:, 0:wn]), op=ALU.mult)
                nc.vector.tensor_tensor(out=a0, in0=a0, in1=a1, op=ALU.add)
                if os.environ.get("BILIN_NO_SCATTER"):
                    nc.sync.dma_start(
                        bass.AP(out_d, 0, [[3, P], [1, wn * 3]]), a0)
                else:
                    nc.gpsimd.indirect_dma_start(
                        out=out_d[:, :],
                        out_offset=bass.IndirectOffsetOnAxis(ap=psct[:, 0:1], axis=0),
                        in_=a0.opt(), in_offset=None)

            prev = None
            for b in range(n_imgs):
                wa, wb_ = wslots[b]
                # ---- image-level: row-pair interleaved copy + zero-fill out
                it = wp.tile([P, 1536], F32, tag="it")
                nc.sync.dma_start(it[:], bass.AP(inp, b * (6 + IMG_ELS) + 6,
                                                 [[1536, P], [1, 1536]]))
                hal = wp.tile([P, ROW_ELS], F32, tag="hal")
                nc.sync.dma_start(hal[0:127, :],
                                  bass.AP(inp, b * (6 + IMG_ELS) + 6 + 1536,
                                          [[1536, 127], [1, ROW_ELS]]))
                nc.sync.dma_start(hal[127:128, :],
                                  bass.AP(inp, b * (6 + IMG_ELS) + 6 + IMG_ELS - ROW_ELS,
                                          [[ROW_ELS, 1], [1, ROW_ELS]]))
                d2 = dp.tile([P, 512, 6], F32, tag="d2")
                nc.vector.tensor_copy(out=d2[:, :, 0:3],
                                      in_=it[:].rearrange("p (w c) -> p w c", c=3))
                nc.vector.tensor_copy(out=d2[:, 0:256, 3:6],
                                      in_=it[:, ROW_ELS:1536].rearrange("p (w c) -> p w c", c=3))
                nc.vector.tensor_copy(out=d2[:, 256:512, 3:6],
                                      in_=hal[:].rearrange("p (w c) -> p w c", c=3))
                nc.sync.dma_start(idups[b][:, :], d2[:])
                # zero-fill this image's output region
                nc.sync.dma_start(
                    bass.AP(out_d, b * IMG_ELS, [[1536, P], [1, 1536]]), zt[:])

                for s, wn in ((0, wa), (1, wb_)):
                    base = (b * 2 + s) * P
                    th4 = wp.tile([P, 4], F32, tag="th4")
                    nc.sync.dma_start(th4[:], bass.AP(th4_d, base * 4, [[4, P], [1, 4]]))
                    sct = wpool.tile([P, 1], I32, tag="sct")
                    nc.sync.dma_start(sct[:], bass.AP(sct_d, base, [[1, P], [1, 1]]))

                    # grid: X = t00*w + A(p), Y = t10*w + B(p)
                    X = wp.tile([P, WMAX], F32, tag="X")
                    nc.vector.tensor_scalar(out=X[:, 0:wn], in0=wio[:, 0:wn],
                                            scalar1=th4[:, 2:3], scalar2=th4[:, 0:1],
                                            op0=ALU.mult, op1=ALU.add)
                    Y = wp.tile([P, WMAX], F32, tag="Y")
                    nc.vector.tensor_scalar(out=Y[:, 0:wn], in0=wio[:, 0:wn],
                                            scalar1=th4[:, 3:4], scalar2=th4[:, 1:2],
                                            op0=ALU.mult, op1=ALU.add)

                    def floor_of(src, nm):
                        ti = wp.tile([P, WMAX], I32, tag=f"fl_i{nm}")
                        nc.vector.tensor_copy(out=ti[:, 0:wn], in_=src[:, 0:wn])
                        tf = wp.tile([P, WMAX], F32, tag=f"fl_f{nm}")
                        nc.vector.tensor_copy(out=tf[:, 0:wn], in_=ti[:, 0:wn])
                        gt = wp.tile([P, WMAX], F32, tag=f"fl_g{nm}")
                        nc.vector.tensor_tensor(out=gt[:, 0:wn], in0=tf[:, 0:wn],
                                                in1=src[:, 0:wn], op=ALU.is_gt)
                        nc.vector.tensor_tensor(out=tf[:, 0:wn], in0=tf[:, 0:wn],
                                                in1=gt[:, 0:wn], op=ALU.subtract)
                        return tf

                    xf = floor_of(X, "x")
                    yf = floor_of(Y, "y")

                    fx = wp.tile([P, WMAX], F32, tag="fx")
                    nc.vector.tensor_tensor(out=fx[:, 0:wn], in0=X[:, 0:wn],
                                            in1=xf[:, 0:wn], op=ALU.subtract)
                    fy = wp.tile([P, WMAX], F32, tag="fy")
                    nc.vector.tensor_tensor(out=fy[:, 0:wn], in0=Y[:, 0:wn],
                                            in1=yf[:, 0:wn], op=ALU.subtract)
                    al = wp.tile([P, WMAX], F32, tag="al")
                    nc.vector.scalar_tensor_tensor(out=al[:, 0:wn], in0=xf[:, 0:wn],
                                                   scalar=1.0, in1=X[:, 0:wn],
                                                   op0=ALU.add, op1=ALU.subtract)
                    ga = wp.tile([P, WMAX], F32, tag="ga")
                    nc.vector.scalar_tensor_tensor(out=ga[:, 0:wn], in0=yf[:, 0:wn],
                                                   scalar=1.0, in1=Y[:, 0:wn],
                                                   op0=ALU.add, op1=ALU.subtract)

                    mx = wp.tile([P, WMAX], F32, tag="mx")
                    nc.vector.tensor_scalar(out=mx[:, 0:wn], in0=xf[:, 0:wn],
                                            scalar1=0.0, scalar2=None, op0=ALU.is_ge)
                    nc.vector.scalar_tensor_tensor(out=mx[:, 0:wn], in0=xf[:, 0:wn],
                                                   scalar=254.0, in1=mx[:, 0:wn],
                                                   op0=ALU.is_le, op1=ALU.mult)
                    my = wp.tile([P, WMAX], F32, tag="my")
                    nc.vector.tensor_scalar(out=my[:, 0:wn], in0=yf[:, 0:wn],
                                            scalar1=0.0, scalar2=None, op0=ALU.is_ge)
                    nc.vector.scalar_tensor_tensor(out=my[:, 0:wn], in0=yf[:, 0:wn],
                                                   scalar=254.0, in1=my[:, 0:wn],
                                                   op0=ALU.is_le, op1=ALU.mult)
                    Aw = wp.tile([P, WMAX], F32, tag="Aw")
                    nc.vector.tensor_tensor(out=Aw[:, 0:wn], in0=al[:, 0:wn],
                                            in1=mx[:, 0:wn], op=ALU.mult)
                    Bw = wp.tile([P, WMAX], F32, tag="Bw")
                    nc.vector.tensor_tensor(out=Bw[:, 0:wn], in0=fx[:, 0:wn],
                                            in1=mx[:, 0:wn], op=ALU.mult)
                    Cw = wp.tile([P, WMAX], F32, tag="Cw")
                    nc.vector.tensor_tensor(out=Cw[:, 0:wn], in0=ga[:, 0:wn],
                                            in1=my[:, 0:wn], op=ALU.mult)
                    Dw = wp.tile([P, WMAX], F32, tag="Dw")
                    nc.vector.tensor_tensor(out=Dw[:, 0:wn], in0=fy[:, 0:wn],
                                            in1=my[:, 0:wn], op=ALU.mult)
                    w00 = wpool.tile([P, WMAX], F32, tag="w00")
                    nc.vector.tensor_tensor(out=w00[:, 0:wn], in0=Cw[:, 0:wn],
                                            in1=Aw[:, 0:wn], op=ALU.mult)
                    w10 = wpool.tile([P, WMAX], F32, tag="w10")
                    nc.vector.tensor_tensor(out=w10[:, 0:wn], in0=Dw[:, 0:wn],
                                            in1=Aw[:, 0:wn], op=ALU.mult)
                    w01 = wpool.tile([P, WMAX], F32, tag="w01")
                    nc.vector.tensor_tensor(out=w01[:, 0:wn], in0=Cw[:, 0:wn],
                                            in1=Bw[:, 0:wn], op=ALU.mult)
                    w11 = wpool.tile([P, WMAX], F32, tag="w11")
                    nc.vector.tensor_tensor(out=w11[:, 0:wn], in0=Dw[:, 0:wn],
                                            in1=Bw[:, 0:wn], op=ALU.mult)

                    xc = wp.tile([P, WMAX], F32, tag="xc")
                    nc.vector.tensor_scalar(out=xc[:, 0:wn], in0=xf[:, 0:wn],
                                            scalar1=0.0, scalar2=254.0,
                                            op0=ALU.max, op1=ALU.min)
                    yc = wp.tile([P, WMAX], F32, tag="yc")
                    nc.vector.tensor_scalar(out=yc[:, 0:wn], in0=yf[:, 0:wn],
                                            scalar1=0.0, scalar2=254.0,
                                            op0=ALU.max, op1=ALU.min)
                    nc.vector.scalar_tensor_tensor(out=yc[:, 0:wn], in0=yc[:, 0:wn],
                                                   scalar=256.0, in1=xc[:, 0:wn],
                                                   op0=ALU.mult, op1=ALU.add)
                    off = wpool.tile([P, WMAX], I32, tag="off")
                    nc.vector.tensor_copy(out=off[:, 0:wn], in_=yc[:, 0:wn])

                    g = gpool.tile([P, WMAX, 12], F32, tag="g")
                    if os.environ.get("BILIN_NO_GATHER"):
                        nc.vector.memset(g[:, 0:wn, :], 0.5)
                    else:
                        for w in range(wn):
                            nc.gpsimd.indirect_dma_start(
                                out=g[:, w, :], out_offset=None,
                                in_=idups[b][:, :],
                                in_offset=bass.IndirectOffsetOnAxis(
                                    ap=off[:, w:w + 1], axis=0))

                    if prev is not None:
                        blend_scatter(prev)
                    prev = (g, w00, w10, w01, w11, sct, wn)

            blend_scatter(prev)
    nc.compile()
    return nc


def _host_plan(inputs):
    """Per image: padded per-row in-bounds j-intervals, 2-pass row packing."""
    B = inputs.shape[0]
    th = inputs[:, :6].reshape(B, 2, 3).astype(np.float32)
    t00, t01, t02 = th[:, 0, 0], th[:, 0, 1], th[:, 0, 2]
    t10, t11, t12 = th[:, 1, 0], th[:, 1, 1], th[:, 1, 2]
    cx = np.float32(127.5) * (t02 + 1.0 - t00 - t01)
    cy = np.float32(127.5) * (t12 + 1.0 - t10 - t11)

    jj = np.arange(W, dtype=np.float32)
    ii = np.arange(H, dtype=np.float32)
    plans = []
    for b in range(B):
        x = (t00[b] * jj[None, :] + t01[b] * ii[:, None] + cx[b]).astype(np.float32)
        y = (t10[b] * jj[None, :] + t11[b] * ii[:, None] + cy[b]).astype(np.float32)
        xf = np.floor(x); yf = np.floor(y)
        inb = (xf >= 0) & (xf <= W - 2) & (yf >= 0) & (yf <= H - 2)   # [H, W]
        has = inb.any(axis=1)
        first = np.where(has, inb.argmax(axis=1), 0)
        last = np.where(has, W - 1 - inb[:, ::-1].argmax(axis=1), -1)
        j_lo = np.maximum(first - PAD, 0)
        j_hi = np.minimum(last + PAD, W - 1)
        ln = np.where(has, j_hi - j_lo + 1, 0)
        order = np.argsort(-ln, kind="stable")
        wA = max(int(ln[order[0]]), 1)
        wB = max(int(ln[order[128]]), 1)
        plans.append(dict(order=order, j_lo=j_lo, ln=ln, wA=wA, wB=wB,
                          t00=t00[b], t01=t01[b], cx=cx[b],
                          t10=t10[b], t11=t11[b], cy=cy[b]))
    return plans


def _core_tensors(plans, img_ids, wslots):
    """Build th4 [(16*2*128), 4] f32 and sct [(16*2*128), 1] i32 for one core."""
    th4 = np.zeros((IMGS * 2 * P, 4), np.float32)
    sct = np.zeros((IMGS * 2 * P, 1), np.int32)
    for k, b in enumerate(img_ids):
        pl = plans[b]
        for s in range(2):
            wn = wslots[k][s]
            rows = pl["order"][s * P:(s + 1) * P]
            dl = np.minimum(pl["j_lo"][rows], W - wn)
            base = (k * 2 + s) * P
            th4[base:base + P, 0] = pl["t00"] * dl + pl["t01"] * rows + pl["cx"]
            th4[base:base + P, 1] = pl["t10"] * dl + pl["t11"] * rows + pl["cy"]
            th4[base:base + P, 2] = pl["t00"]
            th4[base:base + P, 3] = pl["t10"]
            sct[base:base + P, 0] = k * (H * W) + rows * W + dl
    return th4, sct


def kernel(inputs: np.ndarray) -> np.ndarray:
    inputs = np.ascontiguousarray(inputs, dtype=np.float32)
    B = inputs.shape[0]
    assert inputs.shape == (B, 6 + IMG_ELS) and B == N_CORES * IMGS
    plans = _host_plan(inputs)

    # balance: sort images by cost, slot k gets ranks [8k, 8k+8) across cores
    cost = np.array([p["wA"] + p["wB"] for p in plans])
    order = np.argsort(-cost, kind="stable")
    assign = [[0] * IMGS for _ in range(N_CORES)]   # core -> slot -> image id
    wslots = []
    for k in range(IMGS):
        grp = order[k * N_CORES:(k + 1) * N_CORES]
        wA = min(max(int(max(plans[b]["wA"] for b in grp)), 1), WMAX)
        wB = min(max(int(max(plans[b]["wB"] for b in grp)), 1), WMAX)
        wslots.append((wA, wB))
        for c, b in enumerate(grp):
            assign[c][k] = int(b)
    key = tuple(wslots)
    if key not in _cached:
        _cached.clear()
        _cached[key] = _build(key)
    nc = _cached[key]

    wio = np.tile(np.arange(WMAX, dtype=np.float32), (P, 1))
    trace = bool(os.environ.get("BILIN_TRACE"))
    if trace:
        try:
            import antenv.axon_hooks  # noqa: F401
        except ImportError:
            try:
                import types
                from trn_agent_boot.trn_boot import _ntff_profile_via_ctypes
                hook = _ntff_profile_via_ctypes("/opt/axon/libaxon_pjrt.so")
                mod = types.ModuleType("antenv.axon_hooks")
                mod.get_axon_ntff_profile_hook = lambda: hook
                sys.modules["antenv.axon_hooks"] = mod
            except Exception:
                trace = False

    in_maps = []
    for c in range(N_CORES):
        ids = assign[c]
        th4, sct = _core_tensors(plans, ids, wslots)
        in_maps.append(dict(
            inp=np.ascontiguousarray(inputs[ids]),
            wio=wio, th4=th4, sct=sct))
    res = run_bass_kernel_spmd(nc, in_maps, core_ids=list(range(N_CORES)),
                               trace=trace)
    if trace and res.exec_time_ns is not None:
        print(f"HW exec time: {res.exec_time_ns} ns")
    out = np.empty((B, H, W, C), np.float32)
    for c in range(N_CORES):
        ob = res.results[c]["outb"].reshape(IMGS, H, W, C)
        for k, b in enumerate(assign[c]):
            out[b] = ob[k]
    return out


# revision 9
# speedup vs baseline: 3.1898x; 1.5860x over previous
"""Bilinear sampler TRN2 kernel, v2: interval-packed [P,1] gathers.

Key idea: the in-bounds output region (nonzero weights) is the preimage of
the source square under the affine map — per output row i an interval
[j_lo(i), j_hi(i)].  All other pixels are exactly zero in the reference
(clipped-corner weights cancel).  The kernel is value-specialized per call:
the host derives, from the 6 affine params per image (coordinates only, no
image data), a per-row gather window, packs one row per partition with a
per-partition shift, and the device gathers only max_p(len) columns per
pass instead of 512.  Two passes of 128 rows cover the image; results are
scattered back with a single [P,1] indirect DMA per pass into a zero-filled
output.  Per-image instruction count drops ~2.3x vs the dense kernel;
the Pool engine (994ns+ per SWDGE indirect instruction) is the bottleneck.

Layout per pass s of image b: partition p handles output row r(p) =
rows_sorted_by_len[s*128+p], columns j = D(p) + w for w in [0, Wslot).
X = t00*(D+w) + t01*r + cx = t00*w + A(p), A host-computed per partition.
"""
import os
import sys

sys.path.insert(0, "/opt/trn_rl_repo")

import numpy as np

import concourse.bacc as bacc
import concourse.bass as bass
import concourse.mybir as mybir
import concourse.tile as tile
from concourse.bass_utils import run_bass_kernel_spmd

P = 128
H = W = 256
C = 3
IMG_ELS = H * W * C            # 196608
ROW_ELS = W * C                # 768
N_CORES = 8
IMGS = 16
PAD = 2                        # interval padding vs host float rounding
WMAX = 256

F32 = mybir.dt.float32
I32 = mybir.dt.int32
ALU = mybir.AluOpType

_cached = {}


def _build(wslots):
    """wslots: tuple of 16 (W_A, W_B) per image slot."""
    nc = bacc.Bacc("TRN2", target_bir_lowering=False, debug=False,
                   enable_asserts=False, num_devices=1, num_swdge_queues=1)
    n_imgs = len(wslots)
    inp = nc.dram_tensor("inp", [n_imgs, 6 + IMG_ELS], F32, kind="ExternalInput")
    wio_d = nc.dram_tensor("wio", [P, WMAX], F32, kind="ExternalInput")
    th4_d = nc.dram_tensor("th4", [n_imgs * 2 * P, 4], F32, kind="ExternalInput")
    sct_d = nc.dram_tensor("sct", [n_imgs * 2 * P, 1], I32, kind="ExternalInput")
    out_d = nc.dram_tensor("outb", [n_imgs * H * W, C], F32, kind="ExternalOutput")
    idups = [nc.dram_tensor(f"idup{b}", [H * W, 6], F32) for b in range(n_imgs)]

    with tile.TileContext(nc) as tc:
        with (
            tc.tile_pool(name="const", bufs=1) as cpool,
            tc.tile_pool(name="work", bufs=1) as wp,
            tc.tile_pool(name="d2p", bufs=2) as dp,
            tc.tile_pool(name="gath", bufs=3) as gpool,
            tc.tile_pool(name="wgt", bufs=2) as wpool,
            tc.tile_pool(name="outp", bufs=2) as opool,
        ):
            wio = cpool.tile([P, WMAX], F32)
            nc.sync.dma_start(wio[:], wio_d[:, :])
            zt = cpool.tile([P, 1536], F32)
            nc.vector.memset(zt[:], 0.0)

            def bc3(ap):
                return bass.AP(ap.tensor, ap.offset, list(ap.ap) + [[0, 3]])

            def blend_scatter(st):
                pg, pw00, pw10, pw01, pw11, psct, wn = st
                t0 = opool.tile([P, WMAX, 3], F32, tag="bl_t0")
                t1 = opool.tile([P, WMAX, 3], F32, tag="bl_t1")
                a0 = t0[:, 0:wn, :]
                a1 = t1[:, 0:wn, :]
                nc.vector.tensor_tensor(out=a0, in0=pg[:, 0:wn, 0:3],
                                        in1=bc3(pw00[:, 0:wn]), op=ALU.mult)
                nc.vector.tensor_tensor(out=a1, in0=pg[:, 0:wn, 3:6],
                                        in1=bc3(pw10[:, 0:wn]), op=ALU.mult)
                nc.vector.tensor_tensor(out=a0, in0=a0, in1=a1, op=ALU.add)
                nc.vector.tensor_tensor(out=a1, in0=pg[:, 0:wn, 6:9],
                                        in1=bc3(pw01[:, 0:wn]), op=ALU.mult)
                nc.vector.tensor_tensor(out=a0, in0=a0, in1=a1, op=ALU.add)
                nc.vector.tensor_tensor(out=a1, in0=pg[:, 0:wn, 9:12],
                                        in1=bc3(pw11[:, 0:wn]), op=ALU.mult)
                nc.vector.tensor_tensor(out=a0, in0=a0, in1=a1, op=ALU.add)
                if os.environ.get("BILIN_NO_SCATTER"):
                    nc.sync.dma_start(
                        bass.AP(out_d, 0, [[3, P], [1, wn * 3]]), a0)
                else:
                    nc.gpsimd.indirect_dma_start(
                        out=out_d[:, :],
                        out_offset=bass.IndirectOffsetOnAxis(ap=psct[:, 0:1], axis=0),
                        in_=a0.opt(), in_offset=None)

            prev = None
            for b in range(n_imgs):
                wa, wb_ = wslots[b]
                # ---- image-level: row-pair interleaved copy + zero-fill out
                it = wp.tile([P, 1536], F32, tag="it")
                nc.sync.dma_start(it[:], bass.AP(inp, b * (6 + IMG_ELS) + 6,
                                                 [[1536, P], [1, 1536]]))
                hal = wp.tile([P, ROW_ELS], F32, tag="hal")
                nc.sync.dma_start(hal[0:127, :],
                                  bass.AP(inp, b * (6 + IMG_ELS) + 6 + 1536,
                                          [[1536, 127], [1, ROW_ELS]]))
                nc.sync.dma_start(hal[127:128, :],
                                  bass.AP(inp, b * (6 + IMG_ELS) + 6 + IMG_ELS - ROW_ELS,
                                          [[ROW_ELS, 1], [1, ROW_ELS]]))
                d2 = dp.tile([P, 512, 6], F32, tag="d2")
                nc.vector.tensor_copy(out=d2[:, :, 0:3],
                                      in_=it[:].rearrange("p (w c) -> p w c", c=3))
                nc.vector.tensor_copy(out=d2[:, 0:256, 3:6],
                                      in_=it[:, ROW_ELS:1536].rearrange("p (w c) -> p w c", c=3))
                nc.vector.tensor_copy(out=d2[:, 256:512, 3:6],
                                      in_=hal[:].rearrange("p (w c) -> p w c", c=3))
                nc.sync.dma_start(idups[b][:, :], d2[:])
                # zero-fill this image's output region
                nc.sync.dma_start(
                    bass.AP(out_d, b * IMG_ELS, [[1536, P], [1, 1536]]), zt[:])

                for s, wn in ((0, wa), (1, wb_)):
                    base = (b * 2 + s) * P
                    th4 = wp.tile([P, 4], F32, tag="th4")
                    nc.sync.dma_start(th4[:], bass.AP(th4_d, base * 4, [[4, P], [1, 4]]))
                    sct = wpool.tile([P, 1], I32, tag="sct")
                    nc.sync.dma_start(sct[:], bass.AP(sct_d, base, [[1, P], [1, 1]]))

                    # grid: X = t00*w + A(p), Y = t10*w + B(p)
                    X = wp.tile([P, WMAX], F32, tag="X")
                    nc.vector.tensor_scalar(out=X[:, 0:wn], in0=wio[:, 0:wn],
                                            scalar1=th4[:, 2:3], scalar2=th4[:, 0:1],
                                            op0=ALU.mult, op1=ALU.add)
                    Y = wp.tile([P, WMAX], F32, tag="Y")
                    nc.vector.tensor_scalar(out=Y[:, 0:wn], in0=wio[:, 0:wn],
                                            scalar1=th4[:, 3:4], scalar2=th4[:, 1:2],
                                            op0=ALU.mult, op1=ALU.add)

                    def floor_of(src, nm):
                        ti = wp.tile([P, WMAX], I32, tag=f"fl_i{nm}")
                        nc.vector.tensor_copy(out=ti[:, 0:wn], in_=src[:, 0:wn])
                        tf = wp.tile([P, WMAX], F32, tag=f"fl_f{nm}")
                        nc.vector.tensor_copy(out=tf[:, 0:wn], in_=ti[:, 0:wn])
                        gt = wp.tile([P, WMAX], F32, tag=f"fl_g{nm}")
                        nc.vector.tensor_tensor(out=gt[:, 0:wn], in0=tf[:, 0:wn],
                                                in1=src[:, 0:wn], op=ALU.is_gt)
                        nc.vector.tensor_tensor(out=tf[:, 0:wn], in0=tf[:, 0:wn],
                                                in1=gt[:, 0:wn], op=ALU.subtract)
                        return tf

                    xf = floor_of(X, "x")
                    yf = floor_of(Y, "y")

                    fx = wp.tile([P, WMAX], F32, tag="fx")
                    nc.vector.tensor_tensor(out=fx[:, 0:wn], in0=X[:, 0:wn],
                                            in1=xf[:, 0:wn], op=ALU.subtract)
                    fy = wp.tile([P, WMAX], F32, tag="fy")
                    nc.vector.tensor_tensor(out=fy[:, 0:wn], in0=Y[:, 0:wn],
                                            in1=yf[:, 0:wn], op=ALU.subtract)
                    al = wp.tile([P, WMAX], F32, tag="al")
                    nc.vector.scalar_tensor_tensor(out=al[:, 0:wn], in0=xf[:, 0:wn],
                                                   scalar=1.0, in1=X[:, 0:wn],
                                                   op0=ALU.add, op1=ALU.subtract)
                    ga = wp.tile([P, WMAX], F32, tag="ga")
                    nc.vector.scalar_tensor_tensor(out=ga[:, 0:wn], in0=yf[:, 0:wn],
                                                   scalar=1.0, in1=Y[:, 0:wn],
                                                   op0=ALU.add, op1=ALU.subtract)

                    mx = wp.tile([P, WMAX], F32, tag="mx")
                    nc.vector.tensor_scalar(out=mx[:, 0:wn], in0=xf[:, 0:wn],
                                            scalar1=0.0, scalar2=None, op0=ALU.is_ge)
                    nc.vector.scalar_tensor_tensor(out=mx[:, 0:wn], in0=xf[:, 0:wn],
                                                   scalar=254.0, in1=mx[:, 0:wn],
                                                   op0=ALU.is_le, op1=ALU.mult)
                    my = wp.tile([P, WMAX], F32, tag="my")
                    nc.vector.tensor_scalar(out=my[:, 0:wn], in0=yf[:, 0:wn],
                                            scalar1=0.0, scalar2=None, op0=ALU.is_ge)
                    nc.vector.scalar_tensor_tensor(out=my[:, 0:wn], in0=yf[:, 0:wn],
                                                   scalar=254.0, in1=my[:, 0:wn],
                                                   op0=ALU.is_le, op1=ALU.mult)
                    Aw = wp.tile([P, WMAX], F32, tag="Aw")
                    nc.vector.tensor_tensor(out=Aw[:, 0:wn], in0=al[:, 0:wn],
                                            in1=mx[:, 0:wn], op=ALU.mult)
                    Bw = wp.tile([P, WMAX], F32, tag="Bw")
                    nc.vector.tensor_tensor(out=Bw[:, 0:wn], in0=fx[:, 0:wn],
                                            in1=mx[:, 0:wn], op=ALU.mult)
                    Cw = wp.tile([P, WMAX], F32, tag="Cw")
                    nc.vector.tensor_tensor(out=Cw[:, 0:wn], in0=ga[:, 0:wn],
                                            in1=my[:, 0:wn], op=ALU.mult)
                    Dw = wp.tile([P, WMAX], F32, tag="Dw")
                    nc.vector.tensor_tensor(out=Dw[:, 0:wn], in0=fy[:, 0:wn],
                                            in1=my[:, 0:wn], op=ALU.mult)
                    w00 = wpool.tile([P, WMAX], F32, tag="w00")
                    nc.vector.tensor_tensor(out=w00[:, 0:wn], in0=Cw[:, 0:wn],
                                            in1=Aw[:, 0:wn], op=ALU.mult)
                    w10 = wpool.tile([P, WMAX], F32, tag="w10")
                    nc.vector.tensor_tensor(out=w10[:, 0:wn], in0=Dw[:, 0:wn],
                                            in1=Aw[:, 0:wn], op=ALU.mult)
                    w01 = wpool.tile([P, WMAX], F32, tag="w01")
                    nc.vector.tensor_tensor(out=w01[:, 0:wn], in0=Cw[:, 0:wn],
                                            in1=Bw[:, 0:wn], op=ALU.mult)
                    w11 = wpool.tile([P, WMAX], F32, tag="w11")
                    nc.vector.tensor_tensor(out=w11[:, 0:wn], in0=Dw[:, 0:wn],
                                            in1=Bw[:, 0:wn], op=ALU.mult)

                    xc = wp.tile([P, WMAX], F32, tag="xc")
                    nc.vector.tensor_scalar(out=xc[:, 0:wn], in0=xf[:, 0:wn],
                                            scalar1=0.0, scalar2=254.0,
                                            op0=ALU.max, op1=ALU.min)
                    yc = wp.tile([P, WMAX], F32, tag="yc")
                    nc.vector.tensor_scalar(out=yc[:, 0:wn], in0=yf[:, 0:wn],
                                            scalar1=0.0, scalar2=254.0,
                                            op0=ALU.max, op1=ALU.min)
                    nc.vector.scalar_tensor_tensor(out=yc[:, 0:wn], in0=yc[:, 0:wn],
                                                   scalar=256.0, in1=xc[:, 0:wn],
                                                   op0=ALU.mult, op1=ALU.add)
                    off = wpool.tile([P, WMAX], I32, tag="off")
                    nc.vector.tensor_copy(out=off[:, 0:wn], in_=yc[:, 0:wn])

                    g = gpool.tile([P, WMAX, 12], F32, tag="g")
                    if os.environ.get("BILIN_NO_GATHER"):
                        nc.vector.memset(g[:, 0:wn, :], 0.5)
                    else:
                        for w in range(wn):
                            nc.gpsimd.indirect_dma_start(
                                out=g[:, w, :], out_offset=None,
                                in_=idups[b][:, :],
                                in_offset=bass.IndirectOffsetOnAxis(
                                    ap=off[:, w:w + 1], axis=0))

                    if prev is not None:
                        blend_scatter(prev)
                    prev = (g, w00, w10, w01, w11, sct, wn)

            blend_scatter(prev)
    nc.compile()
    return nc


def _host_plan(inputs):
    """Per image: padded per-row in-bounds j-intervals, 2-pass row packing."""
    B = inputs.shape[0]
    th = inputs[:, :6].reshape(B, 2, 3).astype(np.float32)
    t00, t01, t02 = th[:, 0, 0], th[:, 0, 1], th[:, 0, 2]
    t10, t11, t12 = th[:, 1, 0], th[:, 1, 1], th[:, 1, 2]
    cx = np.float32(127.5) * (t02 + 1.0 - t00 - t01)
    cy = np.float32(127.5) * (t12 + 1.0 - t10 - t11)

    jj = np.arange(W, dtype=np.float32)
    ii = np.arange(H, dtype=np.float32)
    plans = []
    for b in range(B):
        x = (t00[b] * jj[None, :] + t01[b] * ii[:, None] + cx[b]).astype(np.float32)
        y = (t10[b] * jj[None, :] + t11[b] * ii[:, None] + cy[b]).astype(np.float32)
        xf = np.floor(x); yf = np.floor(y)
        inb = (xf >= 0) & (xf <= W - 2) & (yf >= 0) & (yf <= H - 2)   # [H, W]
        has = inb.any(axis=1)
        first = np.where(has, inb.argmax(axis=1), 0)
        last = np.where(has, W - 1 - inb[:, ::-1].argmax(axis=1), -1)
        j_lo = np.maximum(first - PAD, 0)
        j_hi = np.minimum(last + PAD, W - 1)
        ln = np.where(has, j_hi - j_lo + 1, 0)
        order = np.argsort(-ln, kind="stable")
        wA = max(int(ln[order[0]]), 1)
        wB = max(int(ln[order[128]]), 1)
        plans.append(dict(order=order, j_lo=j_lo, ln=ln, wA=wA, wB=wB,
                          t00=t00[b], t01=t01[b], cx=cx[b],
                          t10=t10[b], t11=t11[b], cy=cy[b]))
    return plans


def _core_tensors(plans, img_ids, wslots):
    """Build th4 [(16*2*128), 4] f32 and sct [(16*2*128), 1] i32 for one core."""
    th4 = np.zeros((IMGS * 2 * P, 4), np.float32)
    sct = np.zeros((IMGS * 2 * P, 1), np.int32)
    for k, b in enumerate(img_ids):
        pl = plans[b]
        for s in range(2):
            wn = wslots[k][s]
            rows = pl["order"][s * P:(s + 1) * P]
            dl = np.minimum(pl["j_lo"][rows], W - wn)
            base = (k * 2 + s) * P
            th4[base:base + P, 0] = pl["t00"] * dl + pl["t01"] * rows + pl["cx"]
            th4[base:base + P, 1] = pl["t10"] * dl + pl["t11"] * rows + pl["cy"]
            th4[base:base + P, 2] = pl["t00"]
            th4[base:base + P, 3] = pl["t10"]
            sct[base:base + P, 0] = k * (H * W) + rows * W + dl
    return th4, sct


def kernel(inputs: np.ndarray) -> np.ndarray:
    inputs = np.ascontiguousarray(inputs, dtype=np.float32)
    B = inputs.shape[0]
    assert inputs.shape == (B, 6 + IMG_ELS) and B == N_CORES * IMGS
    plans = _host_plan(inputs)

    # balance: group 8 images per slot (one per core); slot cost is
    # max(wA)+max(wB) over the group, so group images with similar (wA, wB).
    # Seed with a wA-sort, then greedy pairwise swaps to shrink component
    # maxes.  Per-core instruction count = sum of slot costs (same for all
    # cores), so minimizing it directly minimizes the Pool-bound runtime.
    wAv = np.array([p["wA"] for p in plans])
    wBv = np.array([p["wB"] for p in plans])
    seed = np.argsort(-wAv, kind="stable")
    groups = [list(seed[k * N_CORES:(k + 1) * N_CORES]) for k in range(IMGS)]

    def gcost(g):
        return int(max(wAv[i] for i in g)) + int(max(wBv[i] for i in g))

    improved = True
    sweeps = 0
    while improved and sweeps < 40:
        improved = False
        sweeps += 1
        for a in range(IMGS):
            for bgrp in range(a + 1, IMGS):
                base_cost = gcost(groups[a]) + gcost(groups[bgrp])
                for i in range(N_CORES):
                    for j in range(N_CORES):
                        ga = groups[a][:]
                        gb = groups[bgrp][:]
                        ga[i], gb[j] = groups[bgrp][j], groups[a][i]
                        if gcost(ga) + gcost(gb) < base_cost:
                            groups[a], groups[bgrp] = ga, gb
                            base_cost = gcost(ga) + gcost(gb)
                            improved = True

    assign = [[0] * IMGS for _ in range(N_CORES)]   # core -> slot -> image id
    wslots = []
    for k in range(IMGS):
        grp = groups[k]
        wA = min(max(int(max(plans[b]["wA"] for b in grp)), 1), WMAX)
        wB = min(max(int(max(plans[b]["wB"] for b in grp)), 1), WMAX)
        wslots.append((wA, wB))
        for c, b in enumerate(grp):
            assign[c][k] = int(b)
    key = tuple(wslots)
    if key not in _cached:
        _cached.clear()
        _cached[key] = _build(key)
    nc = _cached[key]

    wio = np.tile(np.arange(WMAX, dtype=np.float32), (P, 1))
    trace = bool(os.environ.get("BILIN_TRACE"))
    if trace:
        try:
            import antenv.axon_hooks  # noqa: F401
        except ImportError:
            try:
                import types
                from trn_agent_boot.trn_boot import _ntff_profile_via_ctypes
                hook = _ntff_profile_via_ctypes("/opt/axon/libaxon_pjrt.so")
                mod = types.ModuleType("antenv.axon_hooks")
                mod.get_axon_ntff_profile_hook = lambda: hook
                sys.modules["antenv.axon_hooks"] = mod
            except Exception:
                trace = False

    in_maps = []
    for c in range(N_CORES):
        ids = assign[c]
        th4, sct = _core_tensors(plans, ids, wslots)
        in_maps.append(dict(
            inp=np.ascontiguousarray(inputs[ids]),
            wio=wio, th4=th4, sct=sct))
    res = run_bass_kernel_spmd(nc, in_maps, core_ids=list(range(N_CORES)),
                               trace=trace)
    if trace and res.exec_time_ns is not None:
        print(f"HW exec time: {res.exec_time_ns} ns")
    out = np.empty((B, H, W, C), np.float32)
    for c in range(N_CORES):
        ob = res.results[c]["outb"].reshape(IMGS, H, W, C)
        for k, b in enumerate(assign[c]):
            out[b] = ob[k]
    return out


# revision 10
# speedup vs baseline: 3.2233x; 1.0105x over previous
"""Bilinear sampler TRN2 kernel, v3: segment-packed [P,1] gathers.

v2 packed one output row per partition; per-pass width was the max row
interval (max >> mean).  v3 cuts each row's in-bounds interval into
segments of <= KCUT pixels and bin-packs segments of ALL of a core's 16
images into passes of 128 segments, so per-pass width ~= mean segment
length and per-core instruction count approaches sum(len)/128.  Segments
may overlap after left-shifting (window clamp): overlapping pixels compute
identical values, so double-writes are benign.  Passes are decoupled from
images: a single idup table holds all 16 row-pair interleaved images
(gather offsets carry a per-pass image base), idup builds and output
zero-fills all run up front, and each pass scatters to its image's output
region (dummy passes aim at a dump row range).  Slot widths are compiled
per-program as the elementwise max of the 8 cores' sorted pass profiles.
"""
import os
import sys

sys.path.insert(0, "/opt/trn_rl_repo")

import numpy as np

import concourse.bacc as bacc
import concourse.bass as bass
import concourse.mybir as mybir
import concourse.tile as tile
from concourse.bass_utils import run_bass_kernel_spmd

P = 128
H = W = 256
C = 3
IMG_ELS = H * W * C            # 196608
ROW_ELS = W * C                # 768
N_CORES = 8
IMGS = 16
PAD = 2                        # interval padding vs host float rounding
KCUT = 64                      # max segment length at cut time
WMAX = 256

F32 = mybir.dt.float32
I32 = mybir.dt.int32
ALU = mybir.AluOpType

_cached = {}


def _build(wslots):
    """wslots: tuple of per-pass widths (compiled; same for every core)."""
    nc = bacc.Bacc("TRN2", target_bir_lowering=False, debug=False,
                   enable_asserts=False, num_devices=1, num_swdge_queues=1)
    nsl = len(wslots)
    inp = nc.dram_tensor("inp", [IMGS, 6 + IMG_ELS], F32, kind="ExternalInput")
    wio_d = nc.dram_tensor("wio", [P, WMAX], F32, kind="ExternalInput")
    th8_d = nc.dram_tensor("th8", [nsl * P, 8], F32, kind="ExternalInput")
    sct_d = nc.dram_tensor("sct", [nsl * P, 1], I32, kind="ExternalInput")
    # + one dump image region at the end for dummy passes
    out_d = nc.dram_tensor("outb", [(IMGS + 1) * H * W, C], F32,
                           kind="ExternalOutput")
    idup_d = nc.dram_tensor("idup", [IMGS * H * W, 6], F32)

    with tile.TileContext(nc) as tc:
        with (
            tc.tile_pool(name="const", bufs=1) as cpool,
            tc.tile_pool(name="work", bufs=1) as wp,
            tc.tile_pool(name="d2p", bufs=2) as dp,
            tc.tile_pool(name="gath", bufs=3) as gpool,
            tc.tile_pool(name="wgt", bufs=2) as wpool,
            tc.tile_pool(name="outp", bufs=2) as opool,
        ):
            wio = cpool.tile([P, WMAX], F32)
            nc.sync.dma_start(wio[:], wio_d[:, :])
            zt = cpool.tile([P, 1536], F32)
            nc.vector.memset(zt[:], 0.0)

            # ---- up front: all idup builds + all output zero-fills
            for m in range(IMGS):
                it = wp.tile([P, 1536], F32, tag="it")
                nc.sync.dma_start(it[:], bass.AP(inp, m * (6 + IMG_ELS) + 6,
                                                 [[1536, P], [1, 1536]]))
                hal = wp.tile([P, ROW_ELS], F32, tag="hal")
                nc.sync.dma_start(hal[0:127, :],
                                  bass.AP(inp, m * (6 + IMG_ELS) + 6 + 1536,
                                          [[1536, 127], [1, ROW_ELS]]))
                nc.sync.dma_start(hal[127:128, :],
                                  bass.AP(inp, m * (6 + IMG_ELS) + 6 + IMG_ELS - ROW_ELS,
                                          [[ROW_ELS, 1], [1, ROW_ELS]]))
                d2 = dp.tile([P, 512, 6], F32, tag="d2")
                nc.vector.tensor_copy(out=d2[:, :, 0:3],
                                      in_=it[:].rearrange("p (w c) -> p w c", c=3))
                nc.vector.tensor_copy(out=d2[:, 0:256, 3:6],
                                      in_=it[:, ROW_ELS:1536].rearrange("p (w c) -> p w c", c=3))
                nc.vector.tensor_copy(out=d2[:, 256:512, 3:6],
                                      in_=hal[:].rearrange("p (w c) -> p w c", c=3))
                nc.sync.dma_start(
                    bass.AP(idup_d, m * H * W * 6, [[512 * 6, P], [1, 512 * 6]]),
                    d2[:])
                nc.sync.dma_start(
                    bass.AP(out_d, m * IMG_ELS, [[1536, P], [1, 1536]]), zt[:])

            def bc3(ap):
                return bass.AP(ap.tensor, ap.offset, list(ap.ap) + [[0, 3]])

            def blend_scatter(st):
                pg, pw00, pw10, pw01, pw11, psct, wn = st
                t0 = opool.tile([P, WMAX, 3], F32, tag="bl_t0")
                t1 = opool.tile([P, WMAX, 3], F32, tag="bl_t1")
                a0 = t0[:, 0:wn, :]
                a1 = t1[:, 0:wn, :]
                nc.vector.tensor_tensor(out=a0, in0=pg[:, 0:wn, 0:3],
                                        in1=bc3(pw00[:, 0:wn]), op=ALU.mult)
                nc.vector.tensor_tensor(out=a1, in0=pg[:, 0:wn, 3:6],
                                        in1=bc3(pw10[:, 0:wn]), op=ALU.mult)
                nc.vector.tensor_tensor(out=a0, in0=a0, in1=a1, op=ALU.add)
                nc.vector.tensor_tensor(out=a1, in0=pg[:, 0:wn, 6:9],
                                        in1=bc3(pw01[:, 0:wn]), op=ALU.mult)
                nc.vector.tensor_tensor(out=a0, in0=a0, in1=a1, op=ALU.add)
                nc.vector.tensor_tensor(out=a1, in0=pg[:, 0:wn, 9:12],
                                        in1=bc3(pw11[:, 0:wn]), op=ALU.mult)
                nc.vector.tensor_tensor(out=a0, in0=a0, in1=a1, op=ALU.add)
                nc.gpsimd.indirect_dma_start(
                    out=out_d[:, :],
                    out_offset=bass.IndirectOffsetOnAxis(ap=psct[:, 0:1], axis=0),
                    in_=a0.opt(), in_offset=None)

            prev = None
            for k, wn in enumerate(wslots):
                base = k * P
                # th8 cols: 0=A 1=B 2=t00 3=t10 4=gather_base 5..7 pad
                th8 = wp.tile([P, 8], F32, tag="th8")
                nc.sync.dma_start(th8[:], bass.AP(th8_d, base * 8, [[8, P], [1, 8]]))
                sct = wpool.tile([P, 1], I32, tag="sct")
                nc.sync.dma_start(sct[:], bass.AP(sct_d, base, [[1, P], [1, 1]]))

                X = wp.tile([P, WMAX], F32, tag="X")
                nc.vector.tensor_scalar(out=X[:, 0:wn], in0=wio[:, 0:wn],
                                        scalar1=th8[:, 2:3], scalar2=th8[:, 0:1],
                                        op0=ALU.mult, op1=ALU.add)
                Y = wp.tile([P, WMAX], F32, tag="Y")
                nc.vector.tensor_scalar(out=Y[:, 0:wn], in0=wio[:, 0:wn],
                                        scalar1=th8[:, 3:4], scalar2=th8[:, 1:2],
                                        op0=ALU.mult, op1=ALU.add)

                def floor_of(src, nm):
                    ti = wp.tile([P, WMAX], I32, tag=f"fl_i{nm}")
                    nc.vector.tensor_copy(out=ti[:, 0:wn], in_=src[:, 0:wn])
                    tf = wp.tile([P, WMAX], F32, tag=f"fl_f{nm}")
                    nc.vector.tensor_copy(out=tf[:, 0:wn], in_=ti[:, 0:wn])
                    gt = wp.tile([P, WMAX], F32, tag=f"fl_g{nm}")
                    nc.vector.tensor_tensor(out=gt[:, 0:wn], in0=tf[:, 0:wn],
                                            in1=src[:, 0:wn], op=ALU.is_gt)
                    nc.vector.tensor_tensor(out=tf[:, 0:wn], in0=tf[:, 0:wn],
                                            in1=gt[:, 0:wn], op=ALU.subtract)
                    return tf

                xf = floor_of(X, "x")
                yf = floor_of(Y, "y")

                fx = wp.tile([P, WMAX], F32, tag="fx")
                nc.vector.tensor_tensor(out=fx[:, 0:wn], in0=X[:, 0:wn],
                                        in1=xf[:, 0:wn], op=ALU.subtract)
                fy = wp.tile([P, WMAX], F32, tag="fy")
                nc.vector.tensor_tensor(out=fy[:, 0:wn], in0=Y[:, 0:wn],
                                        in1=yf[:, 0:wn], op=ALU.subtract)
                al = wp.tile([P, WMAX], F32, tag="al")
                nc.vector.scalar_tensor_tensor(out=al[:, 0:wn], in0=xf[:, 0:wn],
                                               scalar=1.0, in1=X[:, 0:wn],
                                               op0=ALU.add, op1=ALU.subtract)
                ga = wp.tile([P, WMAX], F32, tag="ga")
                nc.vector.scalar_tensor_tensor(out=ga[:, 0:wn], in0=yf[:, 0:wn],
                                               scalar=1.0, in1=Y[:, 0:wn],
                                               op0=ALU.add, op1=ALU.subtract)

                mx = wp.tile([P, WMAX], F32, tag="mx")
                nc.vector.tensor_scalar(out=mx[:, 0:wn], in0=xf[:, 0:wn],
                                        scalar1=0.0, scalar2=None, op0=ALU.is_ge)
                nc.vector.scalar_tensor_tensor(out=mx[:, 0:wn], in0=xf[:, 0:wn],
                                               scalar=254.0, in1=mx[:, 0:wn],
                                               op0=ALU.is_le, op1=ALU.mult)
                my = wp.tile([P, WMAX], F32, tag="my")
                nc.vector.tensor_scalar(out=my[:, 0:wn], in0=yf[:, 0:wn],
                                        scalar1=0.0, scalar2=None, op0=ALU.is_ge)
                nc.vector.scalar_tensor_tensor(out=my[:, 0:wn], in0=yf[:, 0:wn],
                                               scalar=254.0, in1=my[:, 0:wn],
                                               op0=ALU.is_le, op1=ALU.mult)
                Aw = wp.tile([P, WMAX], F32, tag="Aw")
                nc.vector.tensor_tensor(out=Aw[:, 0:wn], in0=al[:, 0:wn],
                                        in1=mx[:, 0:wn], op=ALU.mult)
                Bw = wp.tile([P, WMAX], F32, tag="Bw")
                nc.vector.tensor_tensor(out=Bw[:, 0:wn], in0=fx[:, 0:wn],
                                        in1=mx[:, 0:wn], op=ALU.mult)
                Cw = wp.tile([P, WMAX], F32, tag="Cw")
                nc.vector.tensor_tensor(out=Cw[:, 0:wn], in0=ga[:, 0:wn],
                                        in1=my[:, 0:wn], op=ALU.mult)
                Dw = wp.tile([P, WMAX], F32, tag="Dw")
                nc.vector.tensor_tensor(out=Dw[:, 0:wn], in0=fy[:, 0:wn],
                                        in1=my[:, 0:wn], op=ALU.mult)
                w00 = wpool.tile([P, WMAX], F32, tag="w00")
                nc.vector.tensor_tensor(out=w00[:, 0:wn], in0=Cw[:, 0:wn],
                                        in1=Aw[:, 0:wn], op=ALU.mult)
                w10 = wpool.tile([P, WMAX], F32, tag="w10")
                nc.vector.tensor_tensor(out=w10[:, 0:wn], in0=Dw[:, 0:wn],
                                        in1=Aw[:, 0:wn], op=ALU.mult)
                w01 = wpool.tile([P, WMAX], F32, tag="w01")
                nc.vector.tensor_tensor(out=w01[:, 0:wn], in0=Cw[:, 0:wn],
                                        in1=Bw[:, 0:wn], op=ALU.mult)
                w11 = wpool.tile([P, WMAX], F32, tag="w11")
                nc.vector.tensor_tensor(out=w11[:, 0:wn], in0=Dw[:, 0:wn],
                                        in1=Bw[:, 0:wn], op=ALU.mult)

                xc = wp.tile([P, WMAX], F32, tag="xc")
                nc.vector.tensor_scalar(out=xc[:, 0:wn], in0=xf[:, 0:wn],
                                        scalar1=0.0, scalar2=254.0,
                                        op0=ALU.max, op1=ALU.min)
                yc = wp.tile([P, WMAX], F32, tag="yc")
                nc.vector.tensor_scalar(out=yc[:, 0:wn], in0=yf[:, 0:wn],
                                        scalar1=0.0, scalar2=254.0,
                                        op0=ALU.max, op1=ALU.min)
                nc.vector.scalar_tensor_tensor(out=yc[:, 0:wn], in0=yc[:, 0:wn],
                                               scalar=256.0, in1=xc[:, 0:wn],
                                               op0=ALU.mult, op1=ALU.add)
                # add per-pass image base into the site index
                nc.vector.tensor_scalar(out=yc[:, 0:wn], in0=yc[:, 0:wn],
                                        scalar1=th8[:, 4:5], scalar2=None,
                                        op0=ALU.add)
                off = wpool.tile([P, WMAX], I32, tag="off")
                nc.vector.tensor_copy(out=off[:, 0:wn], in_=yc[:, 0:wn])

                g = gpool.tile([P, WMAX, 12], F32, tag="g")
                for w in range(wn):
                    nc.gpsimd.indirect_dma_start(
                        out=g[:, w, :], out_offset=None,
                        in_=idup_d[:, :],
                        in_offset=bass.IndirectOffsetOnAxis(
                            ap=off[:, w:w + 1], axis=0))

                if prev is not None:
                    blend_scatter(prev)
                prev = (g, w00, w10, w01, w11, sct, wn)

            blend_scatter(prev)
    nc.compile()
    return nc


def _host_plan(inputs):
    """Per image: padded per-row in-bounds j-intervals."""
    B = inputs.shape[0]
    th = inputs[:, :6].reshape(B, 2, 3).astype(np.float32)
    t00, t01, t02 = th[:, 0, 0], th[:, 0, 1], th[:, 0, 2]
    t10, t11, t12 = th[:, 1, 0], th[:, 1, 1], th[:, 1, 2]
    cx = np.float32(127.5) * (t02 + 1.0 - t00 - t01)
    cy = np.float32(127.5) * (t12 + 1.0 - t10 - t11)

    jj = np.arange(W, dtype=np.float32)
    ii = np.arange(H, dtype=np.float32)
    plans = []
    for b in range(B):
        x = (t00[b] * jj[None, :] + t01[b] * ii[:, None] + cx[b]).astype(np.float32)
        y = (t10[b] * jj[None, :] + t11[b] * ii[:, None] + cy[b]).astype(np.float32)
        xf = np.floor(x); yf = np.floor(y)
        inb = (xf >= 0) & (xf <= W - 2) & (yf >= 0) & (yf <= H - 2)
        has = inb.any(axis=1)
        first = np.where(has, inb.argmax(axis=1), 0)
        last = np.where(has, W - 1 - inb[:, ::-1].argmax(axis=1), -1)
        j_lo = np.maximum(first - PAD, 0)
        j_hi = np.minimum(last + PAD, W - 1)
        ln = np.where(has, j_hi - j_lo + 1, 0)
        plans.append(dict(j_lo=j_lo, ln=ln,
                          t00=float(t00[b]), t01=float(t01[b]), cx=float(cx[b]),
                          t10=float(t10[b]), t11=float(t11[b]), cy=float(cy[b])))
    return plans


def _segments(pl):
    """Cut one image's row intervals into (row, j0, len<=KCUT) segments."""
    segs = []
    for r in range(H):
        ln = int(pl["ln"][r])
        if ln <= 0:
            continue
        j0 = int(pl["j_lo"][r])
        n = -(-ln // KCUT)
        basel = -(-ln // n)
        for i in range(n):
            s0 = j0 + i * basel
            sl = min(basel, j0 + ln - s0)
            segs.append((r, s0, sl))
    return segs


def kernel(inputs: np.ndarray) -> np.ndarray:
    inputs = np.ascontiguousarray(inputs, dtype=np.float32)
    B = inputs.shape[0]
    assert inputs.shape == (B, 6 + IMG_ELS) and B == N_CORES * IMGS
    plans = _host_plan(inputs)
    seg_lists = [_segments(p) for p in plans]
    totals = np.array([sum(s[2] for s in sl) for sl in seg_lists])

    # snake-deal images to cores by total work, exactly IMGS per core
    order = np.argsort(-totals, kind="stable")
    cores = [[] for _ in range(N_CORES)]
    loads = np.zeros(N_CORES)
    for b in order:
        open_cores = [c for c in range(N_CORES) if len(cores[c]) < IMGS]
        c = min(open_cores, key=lambda c: loads[c])
        cores[c].append(int(b))
        loads[c] += totals[b]

    # per-core: all segments (with image slot) sorted by len desc, 128/pass
    core_passes = []
    for c in range(N_CORES):
        segs = []
        for m, b in enumerate(cores[c]):
            if b < 0:
                continue
            segs.extend((sl, m, b, r, j0) for (r, j0, sl) in seg_lists[b])
        segs.sort(key=lambda t: -t[0])
        passes = [segs[i:i + P] for i in range(0, len(segs), P)]
        core_passes.append(passes)

    nsl = max(len(cp) for cp in core_passes)
    wslots = []
    for k in range(nsl):
        wk = 1
        for c in range(N_CORES):
            if k < len(core_passes[c]):
                wk = max(wk, core_passes[c][k][0][0])
        wslots.append(int(min(wk, WMAX)))
    key = tuple(wslots)
    if key not in _cached:
        _cached.clear()
        _cached[key] = _build(key)
    nc = _cached[key]

    DUMP = IMGS * H * W   # out row base of the dump region

    in_maps = []
    for c in range(N_CORES):
        th8 = np.zeros((nsl * P, 8), np.float32)
        sct = np.full((nsl * P, 1), DUMP, np.int32)
        for k in range(nsl):
            wn = wslots[k]
            if k >= len(core_passes[c]):
                continue
            for p, (sl, m, b, r, j0) in enumerate(core_passes[c][k]):
                pl = plans[b]
                j0w = min(j0, W - wn)    # left-shift window to fit row
                row = k * P + p
                th8[row, 0] = pl["t00"] * j0w + pl["t01"] * r + pl["cx"]
                th8[row, 1] = pl["t10"] * j0w + pl["t11"] * r + pl["cy"]
                th8[row, 2] = pl["t00"]
                th8[row, 3] = pl["t10"]
                th8[row, 4] = m * (H * W)
                sct[row, 0] = m * (H * W) + r * W + j0w
        ids = [b if b >= 0 else 0 for b in cores[c]]
        in_maps.append(dict(
            inp=np.ascontiguousarray(inputs[ids]),
            wio=np.tile(np.arange(WMAX, dtype=np.float32), (P, 1)),
            th8=th8, sct=sct))

    trace = bool(os.environ.get("BILIN_TRACE"))
    if trace:
        try:
            import antenv.axon_hooks  # noqa: F401
        except ImportError:
            try:
                import types
                from trn_agent_boot.trn_boot import _ntff_profile_via_ctypes
                hook = _ntff_profile_via_ctypes("/opt/axon/libaxon_pjrt.so")
                mod = types.ModuleType("antenv.axon_hooks")
                mod.get_axon_ntff_profile_hook = lambda: hook
                sys.modules["antenv.axon_hooks"] = mod
            except Exception:
                trace = False

    res = run_bass_kernel_spmd(nc, in_maps, core_ids=list(range(N_CORES)),
                               trace=trace)
    if trace and res.exec_time_ns is not None:
        print(f"HW exec time: {res.exec_time_ns} ns")
    out = np.empty((B, H, W, C), np.float32)
    for c in range(N_CORES):
        ob = res.results[c]["outb"][:DUMP].reshape(IMGS, H, W, C)
        for m, b in enumerate(cores[c]):
            if b >= 0:
                out[b] = ob[m]
    return out


# revision 11
# speedup vs baseline: 3.3105x; 1.0271x over previous
"""Bilinear sampler TRN2 kernel, v3: segment-packed [P,1] gathers.

v2 packed one output row per partition; per-pass width was the max row
interval (max >> mean).  v3 cuts each row's in-bounds interval into
segments of <= KCUT pixels and bin-packs segments of ALL of a core's 16
images into passes of 128 segments, so per-pass width ~= mean segment
length and per-core instruction count approaches sum(len)/128.  Segments
may overlap after left-shifting (window clamp): overlapping pixels compute
identical values, so double-writes are benign.  Passes are decoupled from
images: a single idup table holds all 16 row-pair interleaved images
(gather offsets carry a per-pass image base), idup builds and output
zero-fills all run up front, and each pass scatters to its image's output
region (dummy passes aim at a dump row range).  Slot widths are compiled
per-program as the elementwise max of the 8 cores' sorted pass profiles.
"""
import os
import sys

sys.path.insert(0, "/opt/trn_rl_repo")

import numpy as np

import concourse.bacc as bacc
import concourse.bass as bass
import concourse.mybir as mybir
import concourse.tile as tile
from concourse.bass_utils import run_bass_kernel_spmd

P = 128
H = W = 256
C = 3
IMG_ELS = H * W * C            # 196608
ROW_ELS = W * C                # 768
N_CORES = 8
IMGS = 16
PAD = 2                        # interval padding vs host float rounding
KCUT = 64                      # max segment length at cut time
WMAX = 256

F32 = mybir.dt.float32
I32 = mybir.dt.int32
ALU = mybir.AluOpType

_cached = {}


def _build(wslots):
    """wslots: tuple of per-pass widths (compiled; same for every core)."""
    nc = bacc.Bacc("TRN2", target_bir_lowering=False, debug=False,
                   enable_asserts=False, num_devices=1, num_swdge_queues=1)
    nsl = len(wslots)
    inp = nc.dram_tensor("inp", [IMGS, 6 + IMG_ELS], F32, kind="ExternalInput")
    wio_d = nc.dram_tensor("wio", [P, WMAX], F32, kind="ExternalInput")
    th8_d = nc.dram_tensor("th8", [nsl * P, 8], F32, kind="ExternalInput")
    sct_d = nc.dram_tensor("sct", [nsl * P, 1], I32, kind="ExternalInput")
    # + one dump image region at the end for dummy passes
    out_d = nc.dram_tensor("outb", [(IMGS + 1) * H * W, C], F32,
                           kind="ExternalOutput")
    idup_d = nc.dram_tensor("idup", [IMGS * H * W, 6], F32)

    with tile.TileContext(nc) as tc:
        with (
            tc.tile_pool(name="const", bufs=1) as cpool,
            tc.tile_pool(name="work", bufs=1) as wp,
            tc.tile_pool(name="d2p", bufs=2) as dp,
            tc.tile_pool(name="gath", bufs=3) as gpool,
            tc.tile_pool(name="wgt", bufs=2) as wpool,
            tc.tile_pool(name="outp", bufs=2) as opool,
        ):
            wio = cpool.tile([P, WMAX], F32)
            nc.sync.dma_start(wio[:], wio_d[:, :])
            zt = cpool.tile([P, 1536], F32)
            nc.vector.memset(zt[:], 0.0)

            # ---- up front: all idup builds + all output zero-fills
            # (load tiles live in the double-buffered dp pool so image m+1's
            # loads overlap image m's DVE copies instead of serializing)
            for m in range(IMGS):
                it = dp.tile([P, 1536], F32, tag="it")
                nc.sync.dma_start(it[:], bass.AP(inp, m * (6 + IMG_ELS) + 6,
                                                 [[1536, P], [1, 1536]]))
                hal = dp.tile([P, ROW_ELS], F32, tag="hal")
                nc.sync.dma_start(hal[0:127, :],
                                  bass.AP(inp, m * (6 + IMG_ELS) + 6 + 1536,
                                          [[1536, 127], [1, ROW_ELS]]))
                nc.sync.dma_start(hal[127:128, :],
                                  bass.AP(inp, m * (6 + IMG_ELS) + 6 + IMG_ELS - ROW_ELS,
                                          [[ROW_ELS, 1], [1, ROW_ELS]]))
                d2 = dp.tile([P, 512, 6], F32, tag="d2")
                nc.vector.tensor_copy(out=d2[:, :, 0:3],
                                      in_=it[:].rearrange("p (w c) -> p w c", c=3))
                nc.vector.tensor_copy(out=d2[:, 0:256, 3:6],
                                      in_=it[:, ROW_ELS:1536].rearrange("p (w c) -> p w c", c=3))
                nc.vector.tensor_copy(out=d2[:, 256:512, 3:6],
                                      in_=hal[:].rearrange("p (w c) -> p w c", c=3))
                # idup writes go out on the Activation HWDGE queue so they
                # overlap the next image's loads on the SP queue
                nc.scalar.dma_start(
                    bass.AP(idup_d, m * H * W * 6, [[512 * 6, P], [1, 512 * 6]]),
                    d2[:])
            # zero-fills after the builds: their only deadline is the first
            # scatter (~100us after gathers start), not the first gather
            for m in range(IMGS):
                nc.scalar.dma_start(
                    bass.AP(out_d, m * IMG_ELS, [[1536, P], [1, 1536]]), zt[:])

            def bc3(ap):
                return bass.AP(ap.tensor, ap.offset, list(ap.ap) + [[0, 3]])

            def blend_scatter(st):
                pg, pw00, pw10, pw01, pw11, psct, wn = st
                t0 = opool.tile([P, WMAX, 3], F32, tag="bl_t0")
                t1 = opool.tile([P, WMAX, 3], F32, tag="bl_t1")
                a0 = t0[:, 0:wn, :]
                a1 = t1[:, 0:wn, :]
                nc.vector.tensor_tensor(out=a0, in0=pg[:, 0:wn, 0:3],
                                        in1=bc3(pw00[:, 0:wn]), op=ALU.mult)
                nc.vector.tensor_tensor(out=a1, in0=pg[:, 0:wn, 3:6],
                                        in1=bc3(pw10[:, 0:wn]), op=ALU.mult)
                nc.vector.tensor_tensor(out=a0, in0=a0, in1=a1, op=ALU.add)
                nc.vector.tensor_tensor(out=a1, in0=pg[:, 0:wn, 6:9],
                                        in1=bc3(pw01[:, 0:wn]), op=ALU.mult)
                nc.vector.tensor_tensor(out=a0, in0=a0, in1=a1, op=ALU.add)
                nc.vector.tensor_tensor(out=a1, in0=pg[:, 0:wn, 9:12],
                                        in1=bc3(pw11[:, 0:wn]), op=ALU.mult)
                nc.vector.tensor_tensor(out=a0, in0=a0, in1=a1, op=ALU.add)
                nc.gpsimd.indirect_dma_start(
                    out=out_d[:, :],
                    out_offset=bass.IndirectOffsetOnAxis(ap=psct[:, 0:1], axis=0),
                    in_=a0.opt(), in_offset=None)

            prev = None
            for k, wn in enumerate(wslots):
                base = k * P
                # th8 cols: 0=A 1=B 2=t00 3=t10 4=gather_base 5..7 pad
                th8 = wp.tile([P, 8], F32, tag="th8")
                nc.sync.dma_start(th8[:], bass.AP(th8_d, base * 8, [[8, P], [1, 8]]))
                sct = wpool.tile([P, 1], I32, tag="sct")
                nc.sync.dma_start(sct[:], bass.AP(sct_d, base, [[1, P], [1, 1]]))

                X = wp.tile([P, WMAX], F32, tag="X")
                nc.vector.tensor_scalar(out=X[:, 0:wn], in0=wio[:, 0:wn],
                                        scalar1=th8[:, 2:3], scalar2=th8[:, 0:1],
                                        op0=ALU.mult, op1=ALU.add)
                Y = wp.tile([P, WMAX], F32, tag="Y")
                nc.vector.tensor_scalar(out=Y[:, 0:wn], in0=wio[:, 0:wn],
                                        scalar1=th8[:, 3:4], scalar2=th8[:, 1:2],
                                        op0=ALU.mult, op1=ALU.add)

                def floor_of(src, nm):
                    ti = wp.tile([P, WMAX], I32, tag=f"fl_i{nm}")
                    nc.vector.tensor_copy(out=ti[:, 0:wn], in_=src[:, 0:wn])
                    tf = wp.tile([P, WMAX], F32, tag=f"fl_f{nm}")
                    nc.vector.tensor_copy(out=tf[:, 0:wn], in_=ti[:, 0:wn])
                    gt = wp.tile([P, WMAX], F32, tag=f"fl_g{nm}")
                    nc.vector.tensor_tensor(out=gt[:, 0:wn], in0=tf[:, 0:wn],
                                            in1=src[:, 0:wn], op=ALU.is_gt)
                    nc.vector.tensor_tensor(out=tf[:, 0:wn], in0=tf[:, 0:wn],
                                            in1=gt[:, 0:wn], op=ALU.subtract)
                    return tf

                xf = floor_of(X, "x")
                yf = floor_of(Y, "y")

                fx = wp.tile([P, WMAX], F32, tag="fx")
                nc.vector.tensor_tensor(out=fx[:, 0:wn], in0=X[:, 0:wn],
                                        in1=xf[:, 0:wn], op=ALU.subtract)
                fy = wp.tile([P, WMAX], F32, tag="fy")
                nc.vector.tensor_tensor(out=fy[:, 0:wn], in0=Y[:, 0:wn],
                                        in1=yf[:, 0:wn], op=ALU.subtract)
                al = wp.tile([P, WMAX], F32, tag="al")
                nc.vector.scalar_tensor_tensor(out=al[:, 0:wn], in0=xf[:, 0:wn],
                                               scalar=1.0, in1=X[:, 0:wn],
                                               op0=ALU.add, op1=ALU.subtract)
                ga = wp.tile([P, WMAX], F32, tag="ga")
                nc.vector.scalar_tensor_tensor(out=ga[:, 0:wn], in0=yf[:, 0:wn],
                                               scalar=1.0, in1=Y[:, 0:wn],
                                               op0=ALU.add, op1=ALU.subtract)

                mx = wp.tile([P, WMAX], F32, tag="mx")
                nc.vector.tensor_scalar(out=mx[:, 0:wn], in0=xf[:, 0:wn],
                                        scalar1=0.0, scalar2=None, op0=ALU.is_ge)
                nc.vector.scalar_tensor_tensor(out=mx[:, 0:wn], in0=xf[:, 0:wn],
                                               scalar=254.0, in1=mx[:, 0:wn],
                                               op0=ALU.is_le, op1=ALU.mult)
                my = wp.tile([P, WMAX], F32, tag="my")
                nc.vector.tensor_scalar(out=my[:, 0:wn], in0=yf[:, 0:wn],
                                        scalar1=0.0, scalar2=None, op0=ALU.is_ge)
                nc.vector.scalar_tensor_tensor(out=my[:, 0:wn], in0=yf[:, 0:wn],
                                               scalar=254.0, in1=my[:, 0:wn],
                                               op0=ALU.is_le, op1=ALU.mult)
                Aw = wp.tile([P, WMAX], F32, tag="Aw")
                nc.vector.tensor_tensor(out=Aw[:, 0:wn], in0=al[:, 0:wn],
                                        in1=mx[:, 0:wn], op=ALU.mult)
                Bw = wp.tile([P, WMAX], F32, tag="Bw")
                nc.vector.tensor_tensor(out=Bw[:, 0:wn], in0=fx[:, 0:wn],
                                        in1=mx[:, 0:wn], op=ALU.mult)
                Cw = wp.tile([P, WMAX], F32, tag="Cw")
                nc.vector.tensor_tensor(out=Cw[:, 0:wn], in0=ga[:, 0:wn],
                                        in1=my[:, 0:wn], op=ALU.mult)
                Dw = wp.tile([P, WMAX], F32, tag="Dw")
                nc.vector.tensor_tensor(out=Dw[:, 0:wn], in0=fy[:, 0:wn],
                                        in1=my[:, 0:wn], op=ALU.mult)
                w00 = wpool.tile([P, WMAX], F32, tag="w00")
                nc.vector.tensor_tensor(out=w00[:, 0:wn], in0=Cw[:, 0:wn],
                                        in1=Aw[:, 0:wn], op=ALU.mult)
                w10 = wpool.tile([P, WMAX], F32, tag="w10")
                nc.vector.tensor_tensor(out=w10[:, 0:wn], in0=Dw[:, 0:wn],
                                        in1=Aw[:, 0:wn], op=ALU.mult)
                w01 = wpool.tile([P, WMAX], F32, tag="w01")
                nc.vector.tensor_tensor(out=w01[:, 0:wn], in0=Cw[:, 0:wn],
                                        in1=Bw[:, 0:wn], op=ALU.mult)
                w11 = wpool.tile([P, WMAX], F32, tag="w11")
                nc.vector.tensor_tensor(out=w11[:, 0:wn], in0=Dw[:, 0:wn],
                                        in1=Bw[:, 0:wn], op=ALU.mult)

                xc = wp.tile([P, WMAX], F32, tag="xc")
                nc.vector.tensor_scalar(out=xc[:, 0:wn], in0=xf[:, 0:wn],
                                        scalar1=0.0, scalar2=254.0,
                                        op0=ALU.max, op1=ALU.min)
                yc = wp.tile([P, WMAX], F32, tag="yc")
                nc.vector.tensor_scalar(out=yc[:, 0:wn], in0=yf[:, 0:wn],
                                        scalar1=0.0, scalar2=254.0,
                                        op0=ALU.max, op1=ALU.min)
                nc.vector.scalar_tensor_tensor(out=yc[:, 0:wn], in0=yc[:, 0:wn],
                                               scalar=256.0, in1=xc[:, 0:wn],
                                               op0=ALU.mult, op1=ALU.add)
                # add per-pass image base into the site index
                nc.vector.tensor_scalar(out=yc[:, 0:wn], in0=yc[:, 0:wn],
                                        scalar1=th8[:, 4:5], scalar2=None,
                                        op0=ALU.add)
                off = wpool.tile([P, WMAX], I32, tag="off")
                nc.vector.tensor_copy(out=off[:, 0:wn], in_=yc[:, 0:wn])

                g = gpool.tile([P, WMAX, 12], F32, tag="g")
                for w in range(wn):
                    nc.gpsimd.indirect_dma_start(
                        out=g[:, w, :], out_offset=None,
                        in_=idup_d[:, :],
                        in_offset=bass.IndirectOffsetOnAxis(
                            ap=off[:, w:w + 1], axis=0))

                if prev is not None:
                    blend_scatter(prev)
                prev = (g, w00, w10, w01, w11, sct, wn)

            blend_scatter(prev)
    nc.compile()
    return nc


def _host_plan(inputs):
    """Per image: padded per-row in-bounds j-intervals."""
    B = inputs.shape[0]
    th = inputs[:, :6].reshape(B, 2, 3).astype(np.float32)
    t00, t01, t02 = th[:, 0, 0], th[:, 0, 1], th[:, 0, 2]
    t10, t11, t12 = th[:, 1, 0], th[:, 1, 1], th[:, 1, 2]
    cx = np.float32(127.5) * (t02 + 1.0 - t00 - t01)
    cy = np.float32(127.5) * (t12 + 1.0 - t10 - t11)

    jj = np.arange(W, dtype=np.float32)
    ii = np.arange(H, dtype=np.float32)
    plans = []
    for b in range(B):
        x = (t00[b] * jj[None, :] + t01[b] * ii[:, None] + cx[b]).astype(np.float32)
        y = (t10[b] * jj[None, :] + t11[b] * ii[:, None] + cy[b]).astype(np.float32)
        xf = np.floor(x); yf = np.floor(y)
        inb = (xf >= 0) & (xf <= W - 2) & (yf >= 0) & (yf <= H - 2)
        has = inb.any(axis=1)
        first = np.where(has, inb.argmax(axis=1), 0)
        last = np.where(has, W - 1 - inb[:, ::-1].argmax(axis=1), -1)
        j_lo = np.maximum(first - PAD, 0)
        j_hi = np.minimum(last + PAD, W - 1)
        ln = np.where(has, j_hi - j_lo + 1, 0)
        plans.append(dict(j_lo=j_lo, ln=ln,
                          t00=float(t00[b]), t01=float(t01[b]), cx=float(cx[b]),
                          t10=float(t10[b]), t11=float(t11[b]), cy=float(cy[b])))
    return plans


def _segments(pl):
    """Cut one image's row intervals into (row, j0, len<=KCUT) segments."""
    segs = []
    for r in range(H):
        ln = int(pl["ln"][r])
        if ln <= 0:
            continue
        j0 = int(pl["j_lo"][r])
        n = -(-ln // KCUT)
        basel = -(-ln // n)
        for i in range(n):
            s0 = j0 + i * basel
            sl = min(basel, j0 + ln - s0)
            segs.append((r, s0, sl))
    return segs


def kernel(inputs: np.ndarray) -> np.ndarray:
    inputs = np.ascontiguousarray(inputs, dtype=np.float32)
    B = inputs.shape[0]
    assert inputs.shape == (B, 6 + IMG_ELS) and B == N_CORES * IMGS
    plans = _host_plan(inputs)
    seg_lists = [_segments(p) for p in plans]
    totals = np.array([sum(s[2] for s in sl) for sl in seg_lists])

    # snake-deal images to cores by total work, exactly IMGS per core
    order = np.argsort(-totals, kind="stable")
    cores = [[] for _ in range(N_CORES)]
    loads = np.zeros(N_CORES)
    for b in order:
        open_cores = [c for c in range(N_CORES) if len(cores[c]) < IMGS]
        c = min(open_cores, key=lambda c: loads[c])
        cores[c].append(int(b))
        loads[c] += totals[b]

    # per-core: all segments (with image slot) sorted by len desc, 128/pass
    core_passes = []
    for c in range(N_CORES):
        segs = []
        for m, b in enumerate(cores[c]):
            if b < 0:
                continue
            segs.extend((sl, m, b, r, j0) for (r, j0, sl) in seg_lists[b])
        segs.sort(key=lambda t: -t[0])
        passes = [segs[i:i + P] for i in range(0, len(segs), P)]
        core_passes.append(passes)

    nsl = max(len(cp) for cp in core_passes)
    wslots = []
    for k in range(nsl):
        wk = 1
        for c in range(N_CORES):
            if k < len(core_passes[c]):
                wk = max(wk, core_passes[c][k][0][0])
        wslots.append(int(min(wk, WMAX)))
    key = tuple(wslots)
    if key not in _cached:
        _cached.clear()
        _cached[key] = _build(key)
    nc = _cached[key]

    DUMP = IMGS * H * W   # out row base of the dump region

    in_maps = []
    for c in range(N_CORES):
        th8 = np.zeros((nsl * P, 8), np.float32)
        sct = np.full((nsl * P, 1), DUMP, np.int32)
        for k in range(nsl):
            wn = wslots[k]
            if k >= len(core_passes[c]):
                continue
            for p, (sl, m, b, r, j0) in enumerate(core_passes[c][k]):
                pl = plans[b]
                j0w = min(j0, W - wn)    # left-shift window to fit row
                row = k * P + p
                th8[row, 0] = pl["t00"] * j0w + pl["t01"] * r + pl["cx"]
                th8[row, 1] = pl["t10"] * j0w + pl["t11"] * r + pl["cy"]
                th8[row, 2] = pl["t00"]
                th8[row, 3] = pl["t10"]
                th8[row, 4] = m * (H * W)
                sct[row, 0] = m * (H * W) + r * W + j0w
        ids = [b if b >= 0 else 0 for b in cores[c]]
        in_maps.append(dict(
            inp=np.ascontiguousarray(inputs[ids]),
            wio=np.tile(np.arange(WMAX, dtype=np.float32), (P, 1)),
            th8=th8, sct=sct))

    trace = bool(os.environ.get("BILIN_TRACE"))
    if trace:
        try:
            import antenv.axon_hooks  # noqa: F401
        except ImportError:
            try:
                import types
                from trn_agent_boot.trn_boot import _ntff_profile_via_ctypes
                hook = _ntff_profile_via_ctypes("/opt/axon/libaxon_pjrt.so")
                mod = types.ModuleType("antenv.axon_hooks")
                mod.get_axon_ntff_profile_hook = lambda: hook
                sys.modules["antenv.axon_hooks"] = mod
            except Exception:
                trace = False

    res = run_bass_kernel_spmd(nc, in_maps, core_ids=list(range(N_CORES)),
                               trace=trace)
    if trace and res.exec_time_ns is not None:
        print(f"HW exec time: {res.exec_time_ns} ns")
    out = np.empty((B, H, W, C), np.float32)
    for c in range(N_CORES):
        ob = res.results[c]["outb"][:DUMP].reshape(IMGS, H, W, C)
        for m, b in enumerate(cores[c]):
            if b >= 0:
                out[b] = ob[m]
    return out


# revision 12
# speedup vs baseline: 3.3826x; 1.0218x over previous
"""Bilinear sampler TRN2 kernel, v3: segment-packed [P,1] gathers.

v2 packed one output row per partition; per-pass width was the max row
interval (max >> mean).  v3 cuts each row's in-bounds interval into
segments of <= KCUT pixels and bin-packs segments of ALL of a core's 16
images into passes of 128 segments, so per-pass width ~= mean segment
length and per-core instruction count approaches sum(len)/128.  Segments
may overlap after left-shifting (window clamp): overlapping pixels compute
identical values, so double-writes are benign.  Passes are decoupled from
images: a single idup table holds all 16 row-pair interleaved images
(gather offsets carry a per-pass image base), idup builds and output
zero-fills all run up front, and each pass scatters to its image's output
region (dummy passes aim at a dump row range).  Slot widths are compiled
per-program as the elementwise max of the 8 cores' sorted pass profiles.
"""
import os
import sys

sys.path.insert(0, "/opt/trn_rl_repo")

import numpy as np

import concourse.bacc as bacc
import concourse.bass as bass
import concourse.mybir as mybir
import concourse.tile as tile
from concourse.bass_utils import run_bass_kernel_spmd

P = 128
H = W = 256
C = 3
IMG_ELS = H * W * C            # 196608
ROW_ELS = W * C                # 768
N_CORES = 8
IMGS = 16
PAD = 1                        # interval padding vs host float rounding
KCUT = 64                      # max segment length at cut time
WMAX = 256

F32 = mybir.dt.float32
I32 = mybir.dt.int32
ALU = mybir.AluOpType

_cached = {}


def _build(wslots):
    """wslots: tuple of per-pass widths (compiled; same for every core)."""
    nc = bacc.Bacc("TRN2", target_bir_lowering=False, debug=False,
                   enable_asserts=False, num_devices=1, num_swdge_queues=1)
    nsl = len(wslots)
    inp = nc.dram_tensor("inp", [IMGS, 6 + IMG_ELS], F32, kind="ExternalInput")
    wio_d = nc.dram_tensor("wio", [P, WMAX], F32, kind="ExternalInput")
    th8_d = nc.dram_tensor("th8", [nsl * P, 8], F32, kind="ExternalInput")
    sct_d = nc.dram_tensor("sct", [nsl * P, 1], I32, kind="ExternalInput")
    # + one dump image region at the end for dummy passes
    out_d = nc.dram_tensor("outb", [(IMGS + 1) * H * W, C], F32,
                           kind="ExternalOutput")
    idup_d = nc.dram_tensor("idup", [IMGS * H * W, 6], F32)

    with tile.TileContext(nc) as tc:
        with (
            tc.tile_pool(name="const", bufs=1) as cpool,
            tc.tile_pool(name="work", bufs=1) as wp,
            tc.tile_pool(name="d2p", bufs=2) as dp,
            tc.tile_pool(name="gath", bufs=3) as gpool,
            tc.tile_pool(name="wgt", bufs=2) as wpool,
            tc.tile_pool(name="outp", bufs=2) as opool,
        ):
            wio = cpool.tile([P, WMAX], F32)
            nc.sync.dma_start(wio[:], wio_d[:, :])
            zt = cpool.tile([P, 1536], F32)
            nc.vector.memset(zt[:], 0.0)

            # ---- up front: all idup builds + all output zero-fills
            # (load tiles live in the double-buffered dp pool so image m+1's
            # loads overlap image m's DVE copies instead of serializing)
            for m in range(IMGS):
                ldq = nc.sync if m % 2 == 0 else nc.scalar
                wrq = nc.scalar if m % 2 == 0 else nc.sync
                it = dp.tile([P, 1536], F32, tag="it")
                ldq.dma_start(it[:], bass.AP(inp, m * (6 + IMG_ELS) + 6,
                                             [[1536, P], [1, 1536]]))
                hal = dp.tile([P, ROW_ELS], F32, tag="hal")
                ldq.dma_start(hal[0:127, :],
                              bass.AP(inp, m * (6 + IMG_ELS) + 6 + 1536,
                                      [[1536, 127], [1, ROW_ELS]]))
                ldq.dma_start(hal[127:128, :],
                              bass.AP(inp, m * (6 + IMG_ELS) + 6 + IMG_ELS - ROW_ELS,
                                      [[ROW_ELS, 1], [1, ROW_ELS]]))
                d2 = dp.tile([P, 512, 6], F32, tag="d2")
                nc.vector.tensor_copy(out=d2[:, :, 0:3],
                                      in_=it[:].rearrange("p (w c) -> p w c", c=3))
                nc.vector.tensor_copy(out=d2[:, 0:256, 3:6],
                                      in_=it[:, ROW_ELS:1536].rearrange("p (w c) -> p w c", c=3))
                nc.vector.tensor_copy(out=d2[:, 256:512, 3:6],
                                      in_=hal[:].rearrange("p (w c) -> p w c", c=3))
                # idup write goes out on the opposite HWDGE queue so it
                # overlaps the next image's loads
                wrq.dma_start(
                    bass.AP(idup_d, m * H * W * 6, [[512 * 6, P], [1, 512 * 6]]),
                    d2[:])
            # zero-fills after the builds: their only deadline is the first
            # scatter (~100us after gathers start), not the first gather
            for m in range(IMGS):
                nc.scalar.dma_start(
                    bass.AP(out_d, m * IMG_ELS, [[1536, P], [1, 1536]]), zt[:])

            def bc3(ap):
                return bass.AP(ap.tensor, ap.offset, list(ap.ap) + [[0, 3]])

            def blend_scatter(st):
                pg, pw00, pw10, pw01, pw11, psct, wn = st
                t0 = opool.tile([P, WMAX, 3], F32, tag="bl_t0")
                t1 = opool.tile([P, WMAX, 3], F32, tag="bl_t1")
                a0 = t0[:, 0:wn, :]
                a1 = t1[:, 0:wn, :]
                nc.vector.tensor_tensor(out=a0, in0=pg[:, 0:wn, 0:3],
                                        in1=bc3(pw00[:, 0:wn]), op=ALU.mult)
                nc.vector.tensor_tensor(out=a1, in0=pg[:, 0:wn, 3:6],
                                        in1=bc3(pw10[:, 0:wn]), op=ALU.mult)
                nc.vector.tensor_tensor(out=a0, in0=a0, in1=a1, op=ALU.add)
                nc.vector.tensor_tensor(out=a1, in0=pg[:, 0:wn, 6:9],
                                        in1=bc3(pw01[:, 0:wn]), op=ALU.mult)
                nc.vector.tensor_tensor(out=a0, in0=a0, in1=a1, op=ALU.add)
                nc.vector.tensor_tensor(out=a1, in0=pg[:, 0:wn, 9:12],
                                        in1=bc3(pw11[:, 0:wn]), op=ALU.mult)
                nc.vector.tensor_tensor(out=a0, in0=a0, in1=a1, op=ALU.add)
                nc.gpsimd.indirect_dma_start(
                    out=out_d[:, :],
                    out_offset=bass.IndirectOffsetOnAxis(ap=psct[:, 0:1], axis=0),
                    in_=a0.opt(), in_offset=None)

            prev = None
            for k, wn in enumerate(wslots):
                base = k * P
                # th8 cols: 0=A 1=B 2=t00 3=t10 4=gather_base 5..7 pad
                th8 = wp.tile([P, 8], F32, tag="th8")
                nc.sync.dma_start(th8[:], bass.AP(th8_d, base * 8, [[8, P], [1, 8]]))
                sct = wpool.tile([P, 1], I32, tag="sct")
                nc.sync.dma_start(sct[:], bass.AP(sct_d, base, [[1, P], [1, 1]]))

                X = wp.tile([P, WMAX], F32, tag="X")
                nc.vector.tensor_scalar(out=X[:, 0:wn], in0=wio[:, 0:wn],
                                        scalar1=th8[:, 2:3], scalar2=th8[:, 0:1],
                                        op0=ALU.mult, op1=ALU.add)
                Y = wp.tile([P, WMAX], F32, tag="Y")
                nc.vector.tensor_scalar(out=Y[:, 0:wn], in0=wio[:, 0:wn],
                                        scalar1=th8[:, 3:4], scalar2=th8[:, 1:2],
                                        op0=ALU.mult, op1=ALU.add)

                def floor_of(src, nm):
                    ti = wp.tile([P, WMAX], I32, tag=f"fl_i{nm}")
                    nc.vector.tensor_copy(out=ti[:, 0:wn], in_=src[:, 0:wn])
                    tf = wp.tile([P, WMAX], F32, tag=f"fl_f{nm}")
                    nc.vector.tensor_copy(out=tf[:, 0:wn], in_=ti[:, 0:wn])
                    gt = wp.tile([P, WMAX], F32, tag=f"fl_g{nm}")
                    nc.vector.tensor_tensor(out=gt[:, 0:wn], in0=tf[:, 0:wn],
                                            in1=src[:, 0:wn], op=ALU.is_gt)
                    nc.vector.tensor_tensor(out=tf[:, 0:wn], in0=tf[:, 0:wn],
                                            in1=gt[:, 0:wn], op=ALU.subtract)
                    return tf

                xf = floor_of(X, "x")
                yf = floor_of(Y, "y")

                fx = wp.tile([P, WMAX], F32, tag="fx")
                nc.vector.tensor_tensor(out=fx[:, 0:wn], in0=X[:, 0:wn],
                                        in1=xf[:, 0:wn], op=ALU.subtract)
                fy = wp.tile([P, WMAX], F32, tag="fy")
                nc.vector.tensor_tensor(out=fy[:, 0:wn], in0=Y[:, 0:wn],
                                        in1=yf[:, 0:wn], op=ALU.subtract)
                al = wp.tile([P, WMAX], F32, tag="al")
                nc.vector.scalar_tensor_tensor(out=al[:, 0:wn], in0=xf[:, 0:wn],
                                               scalar=1.0, in1=X[:, 0:wn],
                                               op0=ALU.add, op1=ALU.subtract)
                ga = wp.tile([P, WMAX], F32, tag="ga")
                nc.vector.scalar_tensor_tensor(out=ga[:, 0:wn], in0=yf[:, 0:wn],
                                               scalar=1.0, in1=Y[:, 0:wn],
                                               op0=ALU.add, op1=ALU.subtract)

                mx = wp.tile([P, WMAX], F32, tag="mx")
                nc.vector.tensor_scalar(out=mx[:, 0:wn], in0=xf[:, 0:wn],
                                        scalar1=0.0, scalar2=None, op0=ALU.is_ge)
                nc.vector.scalar_tensor_tensor(out=mx[:, 0:wn], in0=xf[:, 0:wn],
                                               scalar=254.0, in1=mx[:, 0:wn],
                                               op0=ALU.is_le, op1=ALU.mult)
                my = wp.tile([P, WMAX], F32, tag="my")
                nc.vector.tensor_scalar(out=my[:, 0:wn], in0=yf[:, 0:wn],
                                        scalar1=0.0, scalar2=None, op0=ALU.is_ge)
                nc.vector.scalar_tensor_tensor(out=my[:, 0:wn], in0=yf[:, 0:wn],
                                               scalar=254.0, in1=my[:, 0:wn],
                                               op0=ALU.is_le, op1=ALU.mult)
                Aw = wp.tile([P, WMAX], F32, tag="Aw")
                nc.vector.tensor_tensor(out=Aw[:, 0:wn], in0=al[:, 0:wn],
                                        in1=mx[:, 0:wn], op=ALU.mult)
                Bw = wp.tile([P, WMAX], F32, tag="Bw")
                nc.vector.tensor_tensor(out=Bw[:, 0:wn], in0=fx[:, 0:wn],
                                        in1=mx[:, 0:wn], op=ALU.mult)
                Cw = wp.tile([P, WMAX], F32, tag="Cw")
                nc.vector.tensor_tensor(out=Cw[:, 0:wn], in0=ga[:, 0:wn],
                                        in1=my[:, 0:wn], op=ALU.mult)
                Dw = wp.tile([P, WMAX], F32, tag="Dw")
                nc.vector.tensor_tensor(out=Dw[:, 0:wn], in0=fy[:, 0:wn],
                                        in1=my[:, 0:wn], op=ALU.mult)
                w00 = wpool.tile([P, WMAX], F32, tag="w00")
                nc.vector.tensor_tensor(out=w00[:, 0:wn], in0=Cw[:, 0:wn],
                                        in1=Aw[:, 0:wn], op=ALU.mult)
                w10 = wpool.tile([P, WMAX], F32, tag="w10")
                nc.vector.tensor_tensor(out=w10[:, 0:wn], in0=Dw[:, 0:wn],
                                        in1=Aw[:, 0:wn], op=ALU.mult)
                w01 = wpool.tile([P, WMAX], F32, tag="w01")
                nc.vector.tensor_tensor(out=w01[:, 0:wn], in0=Cw[:, 0:wn],
                                        in1=Bw[:, 0:wn], op=ALU.mult)
                w11 = wpool.tile([P, WMAX], F32, tag="w11")
                nc.vector.tensor_tensor(out=w11[:, 0:wn], in0=Dw[:, 0:wn],
                                        in1=Bw[:, 0:wn], op=ALU.mult)

                xc = wp.tile([P, WMAX], F32, tag="xc")
                nc.vector.tensor_scalar(out=xc[:, 0:wn], in0=xf[:, 0:wn],
                                        scalar1=0.0, scalar2=254.0,
                                        op0=ALU.max, op1=ALU.min)
                yc = wp.tile([P, WMAX], F32, tag="yc")
                nc.vector.tensor_scalar(out=yc[:, 0:wn], in0=yf[:, 0:wn],
                                        scalar1=0.0, scalar2=254.0,
                                        op0=ALU.max, op1=ALU.min)
                nc.vector.scalar_tensor_tensor(out=yc[:, 0:wn], in0=yc[:, 0:wn],
                                               scalar=256.0, in1=xc[:, 0:wn],
                                               op0=ALU.mult, op1=ALU.add)
                # add per-pass image base into the site index
                nc.vector.tensor_scalar(out=yc[:, 0:wn], in0=yc[:, 0:wn],
                                        scalar1=th8[:, 4:5], scalar2=None,
                                        op0=ALU.add)
                off = wpool.tile([P, WMAX], I32, tag="off")
                nc.vector.tensor_copy(out=off[:, 0:wn], in_=yc[:, 0:wn])

                g = gpool.tile([P, WMAX, 12], F32, tag="g")
                for w in range(wn):
                    nc.gpsimd.indirect_dma_start(
                        out=g[:, w, :], out_offset=None,
                        in_=idup_d[:, :],
                        in_offset=bass.IndirectOffsetOnAxis(
                            ap=off[:, w:w + 1], axis=0))

                if prev is not None:
                    blend_scatter(prev)
                prev = (g, w00, w10, w01, w11, sct, wn)

            blend_scatter(prev)
    nc.compile()
    return nc


def _host_plan(inputs):
    """Per image: padded per-row in-bounds j-intervals."""
    B = inputs.shape[0]
    th = inputs[:, :6].reshape(B, 2, 3).astype(np.float32)
    t00, t01, t02 = th[:, 0, 0], th[:, 0, 1], th[:, 0, 2]
    t10, t11, t12 = th[:, 1, 0], th[:, 1, 1], th[:, 1, 2]
    cx = np.float32(127.5) * (t02 + 1.0 - t00 - t01)
    cy = np.float32(127.5) * (t12 + 1.0 - t10 - t11)

    jj = np.arange(W, dtype=np.float32)
    ii = np.arange(H, dtype=np.float32)
    plans = []
    for b in range(B):
        x = (t00[b] * jj[None, :] + t01[b] * ii[:, None] + cx[b]).astype(np.float32)
        y = (t10[b] * jj[None, :] + t11[b] * ii[:, None] + cy[b]).astype(np.float32)
        xf = np.floor(x); yf = np.floor(y)
        inb = (xf >= 0) & (xf <= W - 2) & (yf >= 0) & (yf <= H - 2)
        has = inb.any(axis=1)
        first = np.where(has, inb.argmax(axis=1), 0)
        last = np.where(has, W - 1 - inb[:, ::-1].argmax(axis=1), -1)
        j_lo = np.maximum(first - PAD, 0)
        j_hi = np.minimum(last + PAD, W - 1)
        ln = np.where(has, j_hi - j_lo + 1, 0)
        plans.append(dict(j_lo=j_lo, ln=ln,
                          t00=float(t00[b]), t01=float(t01[b]), cx=float(cx[b]),
                          t10=float(t10[b]), t11=float(t11[b]), cy=float(cy[b])))
    return plans


def _segments(pl):
    """Cut one image's row intervals into (row, j0, len<=KCUT) segments."""
    segs = []
    for r in range(H):
        ln = int(pl["ln"][r])
        if ln <= 0:
            continue
        j0 = int(pl["j_lo"][r])
        n = -(-ln // KCUT)
        basel = -(-ln // n)
        for i in range(n):
            s0 = j0 + i * basel
            sl = min(basel, j0 + ln - s0)
            segs.append((r, s0, sl))
    return segs


def kernel(inputs: np.ndarray) -> np.ndarray:
    inputs = np.ascontiguousarray(inputs, dtype=np.float32)
    B = inputs.shape[0]
    assert inputs.shape == (B, 6 + IMG_ELS) and B == N_CORES * IMGS
    plans = _host_plan(inputs)
    seg_lists = [_segments(p) for p in plans]
    totals = np.array([sum(s[2] for s in sl) for sl in seg_lists])

    # snake-deal images to cores by total work, exactly IMGS per core
    order = np.argsort(-totals, kind="stable")
    cores = [[] for _ in range(N_CORES)]
    loads = np.zeros(N_CORES)
    for b in order:
        open_cores = [c for c in range(N_CORES) if len(cores[c]) < IMGS]
        c = min(open_cores, key=lambda c: loads[c])
        cores[c].append(int(b))
        loads[c] += totals[b]

    # per-core: all segments (with image slot) sorted by len desc, 128/pass
    core_passes = []
    for c in range(N_CORES):
        segs = []
        for m, b in enumerate(cores[c]):
            if b < 0:
                continue
            segs.extend((sl, m, b, r, j0) for (r, j0, sl) in seg_lists[b])
        segs.sort(key=lambda t: -t[0])
        passes = [segs[i:i + P] for i in range(0, len(segs), P)]
        core_passes.append(passes)

    nsl = max(len(cp) for cp in core_passes)
    wslots = []
    for k in range(nsl):
        wk = 1
        for c in range(N_CORES):
            if k < len(core_passes[c]):
                wk = max(wk, core_passes[c][k][0][0])
        wslots.append(int(min(wk, WMAX)))
    key = tuple(wslots)
    if key not in _cached:
        _cached.clear()
        _cached[key] = _build(key)
    nc = _cached[key]

    DUMP = IMGS * H * W   # out row base of the dump region

    in_maps = []
    for c in range(N_CORES):
        th8 = np.zeros((nsl * P, 8), np.float32)
        sct = np.full((nsl * P, 1), DUMP, np.int32)
        for k in range(nsl):
            wn = wslots[k]
            if k >= len(core_passes[c]):
                continue
            for p, (sl, m, b, r, j0) in enumerate(core_passes[c][k]):
                pl = plans[b]
                j0w = min(j0, W - wn)    # left-shift window to fit row
                row = k * P + p
                th8[row, 0] = pl["t00"] * j0w + pl["t01"] * r + pl["cx"]
                th8[row, 1] = pl["t10"] * j0w + pl["t11"] * r + pl["cy"]
                th8[row, 2] = pl["t00"]
                th8[row, 3] = pl["t10"]
                th8[row, 4] = m * (H * W)
                sct[row, 0] = m * (H * W) + r * W + j0w
        ids = [b if b >= 0 else 0 for b in cores[c]]
        in_maps.append(dict(
            inp=np.ascontiguousarray(inputs[ids]),
            wio=np.tile(np.arange(WMAX, dtype=np.float32), (P, 1)),
            th8=th8, sct=sct))

    trace = bool(os.environ.get("BILIN_TRACE"))
    if trace:
        try:
            import antenv.axon_hooks  # noqa: F401
        except ImportError:
            try:
                import types
                from trn_agent_boot.trn_boot import _ntff_profile_via_ctypes
                hook = _ntff_profile_via_ctypes("/opt/axon/libaxon_pjrt.so")
                mod = types.ModuleType("antenv.axon_hooks")
                mod.get_axon_ntff_profile_hook = lambda: hook
                sys.modules["antenv.axon_hooks"] = mod
            except Exception:
                trace = False

    res = run_bass_kernel_spmd(nc, in_maps, core_ids=list(range(N_CORES)),
                               trace=trace)
    if trace and res.exec_time_ns is not None:
        print(f"HW exec time: {res.exec_time_ns} ns")
    out = np.empty((B, H, W, C), np.float32)
    for c in range(N_CORES):
        ob = res.results[c]["outb"][:DUMP].reshape(IMGS, H, W, C)
        for m, b in enumerate(cores[c]):
            if b >= 0:
                out[b] = ob[m]
    return out
